# revision 22
# baseline (speedup 1.0000x reference)
"""CKAFormer Trainium2 kernel, fp8 edition.

6 iterations of
    Xn = X / ||X||_row;  P = softmax(relu(Xn@W1+b1)@W2+b2)
    X  = Xn + g*P@(P.T@Xn) - g*Xn@(Xn.T@Xn)
then a final MLP. Row-sharded over 8 NeuronCores.

Speed scheme vs the bf16 baseline:
- State is kept per-row-SCALED (S = nrm*X): the row normalization of the
  leading term cancels. True X is recovered by one in-place scale pass in
  the last iteration only.
- U and V updates accumulate into ONE PSUM bank: with g8 = -8*G,
  er8 = 16*P, ptx8 = 8*PtX and xt8 = 16*Xn^T, both terms come out as
  128*(P@PtX - Xn@G), so a single scalar_tensor_tensor with per-row
  scalar sd*GAMMA/128 applies the whole update.
- Big in-loop matmuls (Gram, V, PtX) run fp8e4m3 DoubleRow (2 k-blocks
  per pass). The U matmuls (K=64) run pairwise-concurrent via
  tile_position row tiling (er8t/ptx8 mirrored to partitions 64-127).
- MLP1 (M=16) runs as 4 concurrent col-tiled chains (col groups 0..3,
  non-DR fp8); MLP2 (K=16) as 4 concurrent row-tiled matmuls. w2b/b1
  are host-replicated across the 4 partition groups.
- Xn^T transposes are produced per-block in the uv(h=0) tail (lag 2)
  right after each block's update+renorm+quantize, so they fill PE slack
  during the DVE-bound uv phase instead of forming a serial phase.
  Iteration 0 interleaves them into the initial norm pass.
- G is AllReduced in fp8 as top-right + bottom-right + top-left [512,512]
  chunks; the bottom-left quadrant is reconstructed locally as
  transpose(top-right). PtX is a fourth fp8 AllReduce. Update error
  enters X only through GAMMA=1e-4.
- Element-wise work is split across DVE and ACT only (GpSimd ucode
  tensor ops are ~10x slower and cannot touch PSUM).
The final MLP also runs fp8, reusing w18 and the tail transposes of the
last iteration's output.
"""

import sys

sys.path.insert(0, "/opt/trn_rl_repo")

import math

import ml_dtypes
import numpy as np

import concourse.bass as bass
import concourse.mybir as mybir
import concourse.tile as tile
from concourse.bass_utils import run_bass_kernel_spmd
from concourse.masks import make_identity
from concourse.vector_clock import ScopedClock

DEPTH = 6
GAMMA = 1e-4
DIM = 1024
HIDDEN = 16
OUT_DIM = 64
N = 16384
CORES = 8

NS = N // CORES        # rows per core = 2048
RT = NS // 128         # row tiles = 16
DK = DIM // 128        # dim k-tiles = 8
P = 128

F32 = mybir.dt.float32
F32R = mybir.dt.float32r
BF = mybir.dt.bfloat16
F8 = mybir.dt.float8e4
AF = mybir.ActivationFunctionType
ALU = mybir.AluOpType
DR = mybir.MatmulPerfMode.DoubleRow

SX = 16.0    # xb8/xt8 = SX * Xn
SW1 = 32.0   # w18 = SW1 * W1
SP = 16.0    # er8/er8t = SP * P
SE = 4.0     # et8 = SE * E
SG = 8.0     # g8 wire = -SG * G ;  ptx8 wire = SG * PtX

GRAM_DRAIN = -SG / (SX * SX)           # psum(SX^2 G) -> -8*G
PTX_DRAIN = SG / (SP * SX)             # psum(SP*SX*PtX) -> 8*PtX
MLP1_SCALE = 1.0 / (SX * SW1)          # psum -> Xn@W1
CUV = GAMMA / (SX * SG)                # fused STT: svc = sd*GAMMA/128

TLAG = 6  # tail transposes run this many blocks behind the uv update

# this container's walrus only accepts one sync-wait slot per engine
# instruction; hoist excess waits onto preceding EventSemaphore carriers.
_MAX_WAITS = 1


class _TC(tile.TileContext):
    def _drain_and_barrier(self, tick_clock, wait_clock):
        drain_inst = self.nc.sync.drain()
        wait_clock.add_sem_waits(
            drain_inst.ins, ScopedClock({None: tick_clock.global_clock})
        )
        si = drain_inst.ins.sync_info
        w = list(si.on_wait) if si and si.on_wait else []
        if len(w) > 1:
            si.on_wait = w[:1]
            for i in range(1, len(w)):
                c = self.nc.sync.drain()
                c.ins.sync_info = mybir.SyncInfo(on_wait=[w[i]], on_update=[])
        self.nc.all_engine_barrier()
        assert self.sems is not None
        popped = self.nc._tile_sem_poison_stack.pop()
        assert popped is self._sem_poison
        self.nc.clear_and_free_semaphores(list(self.sems.allocated().values()))
        self.nc.all_engine_barrier()


def _split_waits(nc, limit=_MAX_WAITS):
    """Hoist excess sem waits onto EventSemaphore carriers inserted just
    before the over-limit instruction (per-engine program order preserves the
    gating; waits are a conjunction so splitting is sound)."""
    nid = 0
    for bb in nc.main_func.blocks:
        out = []
        changed = False
        for ins in bb.instructions:
            si = ins.sync_info
            w = list(si.on_wait) if si and si.on_wait else []
            if len(w) > limit:
                extra, keep = w[:-limit], w[-limit:]
                for i in range(0, len(extra), limit):
                    ev = mybir.InstEventSemaphore(name=f"wsplit_{nid}", ins=[], outs=[])
                    nid += 1
                    ev.engine = ins.engine
                    ev.sync_info = mybir.SyncInfo(
                        on_wait=extra[i : i + limit], on_update=[]
                    )
                    out.append(ev)
                si.on_wait = keep
                changed = True
            out.append(ins)
        if changed:
            bb.instructions = out


def _build():
    nc = bass.Bass()
    x_ext = nc.declare_dram_parameter("x", [NS, DIM], BF, isOutput=False)
    w18_ext = nc.declare_dram_parameter("w18", [P, DK * HIDDEN], F8, isOutput=False)
    w2b4_ext = nc.declare_dram_parameter("w2b4", [P, OUT_DIM], BF, isOutput=False)
    b1s_ext = nc.declare_dram_parameter("b1s", [P, 1], F32, isOutput=False)
    b2_ext = nc.declare_dram_parameter("b2", [OUT_DIM, 1], F32, isOutput=False)
    b2e_ext = nc.declare_dram_parameter("b2e", [OUT_DIM, 1], F32, isOutput=False)
    y_ext = nc.declare_dram_parameter("y", [NS, OUT_DIM], F32, isOutput=True)

    with _TC(nc) as tc:
        with (
            tc.tile_pool(name="state", bufs=1) as st,
            tc.tile_pool(name="sq", bufs=1) as sqp,
            tc.tile_pool(name="stg", bufs=6) as stg,
            tc.tile_pool(name="xtmp", bufs=1) as xtp,
            tc.tile_pool(name="ps", bufs=8, space="PSUM") as ps,
            tc.tile_pool(name="dram", bufs=2, space="DRAM") as dram,
        ):
            # persistent state
            xr = [st.tile([P, DIM], BF, name=f"xr{i}", tag=f"xr{i}") for i in range(RT)]
            xb8 = [st.tile([P, 2, DIM], F8, name=f"xb8{j}", tag=f"xb8{j}") for j in range(DK)]
            xt8 = st.tile([P, DK, NS], F8, name="xt8", tag="xt8")
            g8 = [st.tile([P, 2, DIM], F8, name=f"g8{k}", tag=f"g8{k}") for k in range(DK // 2)]
            et8 = st.tile([OUT_DIM, NS], F8, name="et8", tag="et8")
            er8 = st.tile([P, RT, OUT_DIM], F8, name="er8", tag="er8")
            er8t = st.tile([P, NS], F8, name="er8t", tag="er8t")
            ptx8 = st.tile([P, DIM], F8, name="ptx8", tag="ptx8")
            a1 = st.tile([P, 512], BF, name="a1", tag="a1")
            w18 = st.tile([P, DK * HIDDEN], F8, name="w18", tag="w18")
            w2b4 = st.tile([P, OUT_DIM], BF, name="w2b4", tag="w2b4")
            b1s = st.tile([P, 1], F32, name="b1s", tag="b1s")
            b2 = st.tile([OUT_DIM, 1], F32, name="b2", tag="b2")
            b2e = st.tile([OUT_DIM, 1], F32, name="b2e", tag="b2e")
            ident8 = st.tile([P, P], F8, name="ident8", tag="ident8")
            identf = st.tile([P, P], F32, name="identf", tag="identf")
            # per-iteration stats, double-buffered across iterations
            n2 = [st.tile([P, RT], F32, name=f"n2{s}", tag=f"n2{s}") for s in range(2)]
            sd = [st.tile([P, RT], F32, name=f"sd{s}", tag=f"sd{s}") for s in range(2)]
            inv = [st.tile([P, RT], F32, name=f"inv{s}", tag=f"inv{s}") for s in range(2)]
            sxv = [st.tile([P, RT], F32, name=f"sxv{s}", tag=f"sxv{s}") for s in range(2)]
            srow = [st.tile([P, RT], F32, name=f"srow{s}", tag=f"srow{s}") for s in range(2)]
            s16 = [st.tile([P, RT], F32, name=f"s16{s}", tag=f"s16{s}") for s in range(2)]
            svc = [st.tile([P, RT], F32, name=f"svc{s}", tag=f"svc{s}") for s in range(2)]

            # loads
            for i in range(RT):
                nc.sync.dma_start(xr[i][:], x_ext[i * P : (i + 1) * P, :])
            nc.sync.dma_start(w18[:], w18_ext[:, :])
            nc.sync.dma_start(w2b4[:], w2b4_ext[:, :])
            nc.sync.dma_start(b1s[:], b1s_ext[:, :])
            nc.sync.dma_start(b2[:], b2_ext[:, :])
            nc.sync.dma_start(b2e[:], b2e_ext[:, :])
            make_identity(nc, identf[:])
            nc.vector.tensor_copy(ident8[:], identf[:])

            def norm_stats(i, s):
                # row norm stats of (raw) block i into stats set s:
                # sq+accum, sqrt on ACT; reciprocal, sxv, fused svc on DVE.
                sq = sqp.tile([P, DIM], BF, name="sq", tag="sq")
                nc.scalar.activation(
                    sq[:], xr[i][:], AF.Square, accum_out=n2[s][:, i : i + 1]
                )
                nc.scalar.sqrt(sd[s][:, i : i + 1], n2[s][:, i : i + 1])
                nc.vector.reciprocal(inv[s][:, i : i + 1], sd[s][:, i : i + 1])
                nc.vector.tensor_scalar_mul(
                    sxv[s][:, i : i + 1], inv[s][:, i : i + 1], SX
                )
                nc.vector.tensor_scalar_mul(
                    svc[s][:, i : i + 1], sd[s][:, i : i + 1], CUV
                )

            def quant_block(i, s):
                # xb8 <- fp8(SX * Xn); issued one block late (QLAG) so the
                # engine queues never head-of-line block on the stats chain.
                dst = xb8[i // 2][:, i % 2, :]
                if i % 2 == 0:
                    nc.vector.tensor_scalar_mul(dst, xr[i][:], sxv[s][:, i : i + 1])
                else:
                    nc.scalar.activation(
                        dst, xr[i][:], AF.Copy, scale=sxv[s][:, i : i + 1]
                    )

            def norm_block(i, s):
                norm_stats(i, s)
                quant_block(i, s)

            def transpose_block(b):
                # xt8[:, k, b*128:(b+1)*128] = fp8(SX*Xn[b-block, :].T)
                # 8 stride-2 transposes packed into one PSUM bank, one
                # multi-dim copy out (engine alternates by block parity).
                pt = ps.tile([P, 2048], F8, name="pstb", tag="ps")
                for k in range(DK):
                    nc.tensor.transpose(
                        pt[:, k * 2 * P : (k + 1) * 2 * P : 2],
                        xb8[b // 2][:, b % 2, k * P : (k + 1) * P],
                        ident8[:],
                    )
                dst = xt8[:, :, b * P : (b + 1) * P]
                if b % 2 == 0:
                    nc.scalar.copy(dst, pt[:, 0:2048:2])
                else:
                    nc.vector.tensor_copy(dst, pt[:, 0:2048:2])

            def phase_gram(ms, h, arin, drain_rr, row0=0):
                # partial (SX Xn).T @ (SX Xn) over row tiles for m-blocks `ms`,
                # column half h; drain scaled to -8*G fp8 into arin rows
                # (m-row0)*128.
                for m in ms:
                    pg = ps.tile([P, 512], F32, name="ps", tag="ps")
                    for j in range(DK):
                        nc.tensor.matmul(
                            pg[:],
                            xb8[j][:, :, m * P : (m + 1) * P],
                            xb8[j][:, :, h * 512 : (h + 1) * 512],
                            start=(j == 0),
                            stop=(j == DK - 1),
                            perf_mode=DR,
                        )
                    gs = stg.tile([P, 512], F8, name="gs", tag="gs")
                    if drain_rr.pop(0) == "a":
                        nc.scalar.mul(gs[:], pg[:], GRAM_DRAIN)
                    else:
                        nc.vector.tensor_scalar_mul(gs[:], pg[:], GRAM_DRAIN)
                    nc.sync.dma_start(arin[(m - row0) * P : (m - row0 + 1) * P, :], gs[:])

            def phase_mlp_q(q, pa, et_dst, bias, act_fn):
                # one col-tiled MLP1 chain (M=16 in col group q, non-DR fp8)
                # + its relu, row-tiled MLP2 matmul (K=16 in row group q) and
                # output activation. Chains for different q are independent
                # and run concurrently in the array.
                for j in range(DK):
                    nc.tensor.matmul(
                        pa[32 * q : 32 * q + HIDDEN, :],
                        w18[:, j * HIDDEN : (j + 1) * HIDDEN],
                        xt8[:, j, q * 512 : (q + 1) * 512],
                        start=(j == 0),
                        stop=(j == DK - 1),
                        tile_position=(0, 32 * q),
                    )
                nc.scalar.activation(
                    a1[32 * q : 32 * q + HIDDEN, :],
                    pa[32 * q : 32 * q + HIDDEN, :],
                    AF.Relu,
                    bias=b1s[32 * q : 32 * q + HIDDEN, :],
                    scale=MLP1_SCALE,
                )
                pb = ps.tile([OUT_DIM, 512], F32, name="ps2", tag="ps")
                nc.tensor.matmul(
                    pb[:],
                    w2b4[32 * q : 32 * q + HIDDEN, :],
                    a1[32 * q : 32 * q + HIDDEN, :],
                    tile_position=(32 * q, 0),
                )
                sl = slice(q * 512, (q + 1) * 512)
                nc.scalar.activation(et_dst[:, sl], pb[:], act_fn, bias=bias[:])

            def phase_et_transpose(s):
                # transpose et8 (4E) to rows (stride-2 fp8); srow = sum(4E)
                pts = []
                for j2 in range(2):
                    pt = ps.tile([P, 1024], F8, name="ps8", tag="ps")
                    for q in range(8):
                        i = 8 * j2 + q
                        nc.tensor.transpose(
                            pt[:, q * 2 * OUT_DIM : (q + 1) * 2 * OUT_DIM : 2],
                            et8[:, i * P : (i + 1) * P],
                            ident8[:OUT_DIM, :OUT_DIM],
                        )
                    nc.vector.tensor_reduce(
                        srow[s][:, 8 * j2 : 8 * j2 + 8],
                        pt[:].rearrange("p (i o t) -> p i o t", o=OUT_DIM, t=2)[:, :, :, 0],
                        mybir.AxisListType.X,
                        ALU.add,
                    )
                    pts.append(pt)
                return pts

            def phase_p(pts, s):
                # er8[:, i, :] = fp8(SP * P-rows) = pt * s16 (split ACT/DVE)
                nc.vector.reciprocal(s16[s][:], srow[s][:])
                nc.vector.tensor_scalar_mul(s16[s][:], s16[s][:], SP)
                for j2 in range(2):
                    for q in range(8):
                        i = 8 * j2 + q
                        src = pts[j2][:, q * 2 * OUT_DIM : (q + 1) * 2 * OUT_DIM : 2]
                        if i % 2 == 0:
                            nc.vector.tensor_scalar_mul(
                                er8[:, i, :], src, s16[s][:, i : i + 1]
                            )
                        else:
                            nc.scalar.mul(
                                er8[:, i, :], src, s16[s][:, i : i + 1]
                            )

            def phase_er8t():
                # er8t = (16P).T via PE transposes of er8 rows; mirrored to
                # partitions 64-127 with one SBUF->SBUF DMA for U row-tiling.
                for j2 in range(2):
                    pt = ps.tile([OUT_DIM, 2048], F8, name="pse", tag="ps")
                    for q in range(8):
                        i = 8 * j2 + q
                        nc.tensor.transpose(
                            pt[:, q * 2 * P : (q + 1) * 2 * P : 2],
                            er8[:, i, :],
                            ident8[:],
                        )
                    if j2 == 0:
                        nc.scalar.copy(
                            er8t[:OUT_DIM, j2 * 1024 : (j2 + 1) * 1024], pt[:, 0:2048:2]
                        )
                    else:
                        nc.vector.tensor_copy(
                            er8t[:OUT_DIM, j2 * 1024 : (j2 + 1) * 1024], pt[:, 0:2048:2]
                        )
                nc.sync.dma_start(er8t[OUT_DIM : 2 * OUT_DIM, :], er8t[:OUT_DIM, :])

            def phase_ptx(arin):
                # partial (SP*P).T @ (SX*Xn) -> fp8(SG*PtX) wire
                for h in range(2):
                    pp = ps.tile([OUT_DIM, 512], F32, name="ps", tag="ps")
                    for j in range(DK):
                        nc.tensor.matmul(
                            pp[:],
                            er8[:, 2 * j : 2 * j + 2, :],
                            xb8[j][:, :, h * 512 : (h + 1) * 512],
                            start=(j == 0),
                            stop=(j == DK - 1),
                            perf_mode=DR,
                        )
                    pps = stg.tile([OUT_DIM, 512], F8, name="pps", tag="gs")
                    nc.scalar.mul(pps[:], pp[:], PTX_DRAIN)
                    nc.sync.dma_start(arin[:, h * 512 : (h + 1) * 512], pps[:])

            def phase_bl():
                # bottom-left of g8 = transpose(top-right): g8 cols 0:512 for
                # k-blocks 4..7 from g8 cols 512:1024 of k-blocks 0..3.
                for b in range(4):
                    pt = ps.tile([P, 1024], F8, name="ps8", tag="ps")
                    for a in range(4):
                        nc.tensor.transpose(
                            pt[:, a * 2 * P : (a + 1) * 2 * P : 2],
                            g8[a // 2][:, a % 2, 512 + b * P : 512 + (b + 1) * P],
                            ident8[:],
                        )
                    if b % 2 == 0:
                        nc.scalar.copy(g8[2 + b // 2][:, b % 2, 0:512], pt[:, 0:1024:2])
                    else:
                        nc.vector.tensor_copy(
                            g8[2 + b // 2][:, b % 2, 0:512], pt[:, 0:1024:2]
                        )

            def phase_uv(h, s, tail=None):
                # per block: fused PSUM chain 128*(P@PtX - Xn@G) cols h via
                # 4 DR matmuls + a U matmul; U matmuls run pairwise
                # concurrent via row tiles (0,0)/(64,0) using the er8t/ptx8
                # partition mirrors. Then one STT: xsl += svc * psum.
                pus = {}
                def u_and_stt(i, tp):
                    pu = pus.pop(i)
                    nc.tensor.matmul(
                        pu[:],
                        er8t[tp : tp + OUT_DIM, i * P : (i + 1) * P],
                        ptx8[tp : tp + OUT_DIM, h * 512 : (h + 1) * 512],
                        start=False,
                        stop=True,
                        tile_position=(tp, 0),
                    )
                    return pu
                for i in range(RT):
                    pu = ps.tile([P, 512], F32, name="ps", tag="ps")
                    pus[i] = pu
                    for kk in range(DK // 2):
                        nc.tensor.matmul(
                            pu[:],
                            xt8[:, 2 * kk : 2 * kk + 2, i * P : (i + 1) * P],
                            g8[kk][:, :, h * 512 : (h + 1) * 512],
                            start=(kk == 0),
                            stop=False,
                            perf_mode=DR,
                        )
                    if i % 2 == 1:
                        pua = u_and_stt(i - 1, 0)
                        pub = u_and_stt(i, OUT_DIM)
                        for ii, pu_ in ((i - 1, pua), (i, pub)):
                            xsl = xr[ii][:, h * 512 : (h + 1) * 512]
                            nc.vector.scalar_tensor_tensor(
                                xsl, pu_[:], svc[s][:, ii : ii + 1], xsl,
                                ALU.mult, ALU.add,
                            )
                            if tail is not None:
                                tail(ii)

            rg = [list(range(CORES))]
            # initial norm pass with iter-0 transposes interleaved (PE is
            # otherwise idle while ACT/DVE normalize).
            for i in range(RT):
                norm_block(i, 0)
                if i >= TLAG:
                    transpose_block(i - TLAG)
            for b in range(RT - TLAG, RT):
                transpose_block(b)

            for it in range(DEPTH):
                s = it % 2
                arin_tr = dram.tile([512, 512], F8, name="arin_tr", tag="arin_tr")
                arout_tr = dram.tile([512, 512], F8, name="arout_tr", tag="arout_tr", addr_space="Shared")
                arin_br = dram.tile([512, 512], F8, name="arin_br", tag="arin_br")
                arout_br = dram.tile([512, 512], F8, name="arout_br", tag="arout_br", addr_space="Shared")
                arin_tl = dram.tile([512, 512], F8, name="arin_tl", tag="arin_tl")
                arout_tl = dram.tile([512, 512], F8, name="arout_tl", tag="arout_tl", addr_space="Shared")
                arin_p = dram.tile([OUT_DIM, DIM], F8, name="arin_p", tag="arin_p")
                arout_p = dram.tile([OUT_DIM, DIM], F8, name="arout_p", tag="arout_p", addr_space="Shared")

                drains = list("avavavav")
                phase_gram(range(DK // 2), 1, arin_tr, drains[:4], row0=0)
                nc.gpsimd.collective_compute(
                    "AllReduce", ALU.add,
                    ins=[arin_tr.opt()], outs=[arout_tr.opt()], replica_groups=rg,
                )
                phase_gram(range(DK // 2, DK), 1, arin_br, drains[4:], row0=DK // 2)
                nc.gpsimd.collective_compute(
                    "AllReduce", ALU.add,
                    ins=[arin_br.opt()], outs=[arout_br.opt()], replica_groups=rg,
                )
                # tl gram (AllReduce emitted AFTER the PtX one: uv h=1's U
                # matmuls need p mid-loop; tl is only needed at uv h=0),
                # interleaved with this iteration's transposes and the
                # per-q mlp chains. The mix of hot DR work and cool
                # transposes sits in the AllReduce latency shadow and keeps
                # the PE activity steady; each mlp chain q only needs
                # transpose blocks 4q..4q+3.
                pa = ps.tile([P, 512], F32, name="ps", tag="ps")
                for q in range(4):
                    phase_gram([q], 0, arin_tl, ["a" if q % 2 else "v"])
                    if it > 0:
                        for b in range(4 * q, 4 * q + 4):
                            transpose_block(b)
                    phase_mlp_q(q, pa, et8, b2e, AF.Exp)
                pts = phase_et_transpose(s)
                phase_p(pts, s)
                phase_ptx(arin_p)
                nc.gpsimd.collective_compute(
                    "AllReduce", ALU.add,
                    ins=[arin_p.opt()], outs=[arout_p.opt()], replica_groups=rg,
                )
                nc.gpsimd.collective_compute(
                    "AllReduce", ALU.add,
                    ins=[arin_tl.opt()], outs=[arout_tl.opt()], replica_groups=rg,
                )
                phase_er8t()
                # land AllReduce results
                for k in range(DK // 2):
                    nc.sync.dma_start(
                        g8[k // 2][:, k % 2, 512:1024],
                        arout_tr[k * P : (k + 1) * P, :],
                    )
                for k in range(DK // 2, DK):
                    nc.sync.dma_start(
                        g8[k // 2][:, k % 2, 512:1024],
                        arout_br[(k - DK // 2) * P : (k - DK // 2 + 1) * P, :],
                    )
                for k in range(DK // 2):
                    nc.sync.dma_start(
                        g8[k // 2][:, k % 2, 0:512],
                        arout_tl[k * P : (k + 1) * P, :],
                    )
                nc.sync.dma_start(ptx8[:OUT_DIM, :], arout_p[:, :])
                nc.sync.dma_start(ptx8[OUT_DIM : 2 * OUT_DIM, :], arout_p[:, :])
                phase_bl()
                phase_uv(1, s)
                so = (it + 1) % 2
                if it < DEPTH - 1:
                    def tail(i, it=it, s=s, so=so):
                        norm_stats(i, so)
                        if i >= 1:
                            quant_block(i - 1, so)
                    phase_uv(0, s, tail=tail)
                    quant_block(RT - 1, so)
                else:
                    # fb8 = fp8(SX * X_6) = fp8(S * SX*inv) into xb8 tiles
                    # (X_6 = S_6 * inv_5; xb8 is dead after this iter's
                    # gram/ptx so the tiles are recycled for the final MLP)
                    def fquant(i, s=s, so=so):
                        dst = xb8[i // 2][:, i % 2, :]
                        if i % 2 == 0:
                            nc.vector.tensor_scalar_mul(
                                dst, xr[i][:], sxv[so][:, i : i + 1]
                            )
                        else:
                            nc.scalar.activation(
                                dst, xr[i][:], AF.Copy,
                                scale=sxv[so][:, i : i + 1],
                            )
                    def tail(i, it=it, s=s, so=so):
                        nc.vector.tensor_scalar_mul(
                            sxv[so][:, i : i + 1], inv[s][:, i : i + 1], SX
                        )
                        if i >= 1:
                            fquant(i - 1)
                    phase_uv(0, s, tail=tail)
                    fquant(RT - 1)

            # final MLP in fp8 on the final state's transposes
            yt = xtp.tile([OUT_DIM, NS], F32, name="yt", tag="yt")
            pa = ps.tile([P, 512], F32, name="ps", tag="ps")
            for q in range(4):
                for b in range(4 * q, 4 * q + 4):
                    transpose_block(b)
                phase_mlp_q(q, pa, yt, b2, AF.Identity)
            # transpose Y.T -> rows and store
            yr = sqp.tile([P, RT, OUT_DIM], F32, name="yr", tag="sq")
            for j2 in range(2):
                pt = ps.tile([P, 512], F32, name="ps", tag="ps")
                for q in range(8):
                    i = 8 * j2 + q
                    nc.tensor.transpose(
                        pt[:, q * OUT_DIM : (q + 1) * OUT_DIM],
                        yt[:, i * P : (i + 1) * P].bitcast(F32),
                        identf[:OUT_DIM, :OUT_DIM],
                    )
                nc.vector.tensor_copy(yr[:, 8 * j2 : 8 * j2 + 8, :], pt[:])
            nc.sync.dma_start(
                y_ext.rearrange("(i p) o -> p i o", p=P), yr[:, :, :]
            )

    _split_waits(nc)
    return nc


_NC = None


def _get_nc():
    global _NC
    if _NC is None:
        _NC = _build()
    return _NC


def _q8(x):
    return np.clip(x, -240.0, 240.0).astype(ml_dtypes.float8_e4m3)


def _in_maps(X, W1, b1, W2, b2):
    X = np.asarray(X, dtype=np.float32)
    W1 = np.asarray(W1, dtype=np.float32)
    b1 = np.asarray(b1, dtype=np.float32).reshape(HIDDEN)
    W2 = np.asarray(W2, dtype=np.float32)
    b2c = np.asarray(b2, dtype=np.float32).reshape(OUT_DIM, 1)
    b2e = b2c + np.float32(math.log(SE))
    # w18: fp8(SW1*W1) packed [128, j, h] flat (non-interleaved; MLP1 runs
    # non-DR col-tiled)
    w18 = np.zeros((P, DK * HIDDEN), np.float32)
    for j in range(DK):
        w18[:, j * HIDDEN : (j + 1) * HIDDEN] = W1[j * P : (j + 1) * P, :] * SW1
    w18 = _q8(w18)
    # w2b4/b1s: replicated into partition groups 32q..32q+15 for the
    # col/row-tiled MLP
    w2b4 = np.zeros((P, OUT_DIM), np.float32)
    b1s = np.zeros((P, 1), np.float32)
    for q in range(4):
        w2b4[32 * q : 32 * q + HIDDEN, :] = W2
        b1s[32 * q : 32 * q + HIDDEN, 0] = b1
    w2b4 = w2b4.astype(ml_dtypes.bfloat16)
    Xb = X.astype(ml_dtypes.bfloat16)
    return [
        {
            "x": np.ascontiguousarray(Xb[c * NS : (c + 1) * NS]),
            "w18": w18,
            "w2b4": w2b4,
            "b1s": b1s,
            "b2": b2c,
            "b2e": b2e,
        }
        for c in range(CORES)
    ]


def run(X, W1, b1, W2, b2, **kwargs):
    nc = _get_nc()
    res = run_bass_kernel_spmd(nc, _in_maps(X, W1, b1, W2, b2), list(range(CORES)), **kwargs)
    out = np.concatenate([res.results[c]["y"] for c in range(CORES)], axis=0)
    return out, res


def kernel(X, W1, b1, W2, b2):
    out, _ = run(X, W1, b1, W2, b2)
    return out


# revision 25
# speedup vs baseline: 1.0517x; 1.0517x over previous
"""CKAFormer Trainium2 kernel, fp8 edition.

6 iterations of
    Xn = X / ||X||_row;  P = softmax(relu(Xn@W1+b1)@W2+b2)
    X  = Xn + g*P@(P.T@Xn) - g*Xn@(Xn.T@Xn)
then a final MLP. Row-sharded over 8 NeuronCores.

Speed scheme vs the bf16 baseline:
- State is kept per-row-SCALED (S = nrm*X): the row normalization of the
  leading term cancels. True X is recovered by one in-place scale pass in
  the last iteration only.
- U and V updates accumulate into ONE PSUM bank: with g8 = -8*G,
  er8 = 16*P, ptx8 = 8*PtX and xt8 = 16*Xn^T, both terms come out as
  128*(P@PtX - Xn@G), so a single scalar_tensor_tensor with per-row
  scalar sd*GAMMA/128 applies the whole update.
- Big in-loop matmuls (Gram, V, PtX) run fp8e4m3 DoubleRow (2 k-blocks
  per pass). The U matmuls (K=64) run pairwise-concurrent via
  tile_position row tiling (er8t/ptx8 mirrored to partitions 64-127).
- MLP1 (M=16) runs as 4 concurrent col-tiled chains (col groups 0..3,
  non-DR fp8); MLP2 (K=16) as 4 concurrent row-tiled matmuls. w2b/b1
  are host-replicated across the 4 partition groups.
- Xn^T transposes are produced per-block in the uv(h=0) tail (lag 2)
  right after each block's update+renorm+quantize, so they fill PE slack
  during the DVE-bound uv phase instead of forming a serial phase.
  Iteration 0 interleaves them into the initial norm pass.
- G is AllReduced in fp8 as top-right + bottom-right + top-left [512,512]
  chunks; the bottom-left quadrant is reconstructed locally as
  transpose(top-right). PtX is a fourth fp8 AllReduce. Update error
  enters X only through GAMMA=1e-4.
- Element-wise work is split across DVE and ACT only (GpSimd ucode
  tensor ops are ~10x slower and cannot touch PSUM).
The final MLP also runs fp8, reusing w18 and the tail transposes of the
last iteration's output.
"""

import sys

sys.path.insert(0, "/opt/trn_rl_repo")

import math

import ml_dtypes
import numpy as np

import concourse.bass as bass
import concourse.mybir as mybir
import concourse.tile as tile
from concourse.bass_utils import run_bass_kernel_spmd
from concourse.masks import make_identity
from concourse.vector_clock import ScopedClock

DEPTH = 6
GAMMA = 1e-4
DIM = 1024
HIDDEN = 16
OUT_DIM = 64
N = 16384
CORES = 8

NS = N // CORES        # rows per core = 2048
RT = NS // 128         # row tiles = 16
DK = DIM // 128        # dim k-tiles = 8
P = 128

F32 = mybir.dt.float32
F32R = mybir.dt.float32r
BF = mybir.dt.bfloat16
F8 = mybir.dt.float8e4
AF = mybir.ActivationFunctionType
ALU = mybir.AluOpType
DR = mybir.MatmulPerfMode.DoubleRow

SX = 16.0    # xb8/xt8 = SX * Xn
SW1 = 32.0   # w18 = SW1 * W1
SP = 16.0    # er8/er8t = SP * P
SE = 4.0     # et8 = SE * E
SG = 8.0     # g8 wire = -SG * G ;  ptx8 wire = SG * PtX

GRAM_DRAIN = -SG / (SX * SX)           # psum(SX^2 G) -> -8*G
PTX_DRAIN = SG / (SP * SX)             # psum(SP*SX*PtX) -> 8*PtX
MLP1_SCALE = 1.0 / (SX * SW1)          # psum -> Xn@W1
CUV = GAMMA / (SX * SG)                # fused STT: svc = sd*GAMMA/128

TLAG = 6  # tail transposes run this many blocks behind the uv update

# this container's walrus only accepts one sync-wait slot per engine
# instruction; hoist excess waits onto preceding EventSemaphore carriers.
_MAX_WAITS = 1


class _TC(tile.TileContext):
    def _drain_and_barrier(self, tick_clock, wait_clock):
        drain_inst = self.nc.sync.drain()
        wait_clock.add_sem_waits(
            drain_inst.ins, ScopedClock({None: tick_clock.global_clock})
        )
        si = drain_inst.ins.sync_info
        w = list(si.on_wait) if si and si.on_wait else []
        if len(w) > 1:
            si.on_wait = w[:1]
            for i in range(1, len(w)):
                c = self.nc.sync.drain()
                c.ins.sync_info = mybir.SyncInfo(on_wait=[w[i]], on_update=[])
        self.nc.all_engine_barrier()
        assert self.sems is not None
        popped = self.nc._tile_sem_poison_stack.pop()
        assert popped is self._sem_poison
        self.nc.clear_and_free_semaphores(list(self.sems.allocated().values()))
        self.nc.all_engine_barrier()


def _split_waits(nc, limit=_MAX_WAITS):
    """Hoist excess sem waits onto EventSemaphore carriers inserted just
    before the over-limit instruction (per-engine program order preserves the
    gating; waits are a conjunction so splitting is sound)."""
    nid = 0
    for bb in nc.main_func.blocks:
        out = []
        changed = False
        for ins in bb.instructions:
            si = ins.sync_info
            w = list(si.on_wait) if si and si.on_wait else []
            if len(w) > limit:
                extra, keep = w[:-limit], w[-limit:]
                for i in range(0, len(extra), limit):
                    ev = mybir.InstEventSemaphore(name=f"wsplit_{nid}", ins=[], outs=[])
                    nid += 1
                    ev.engine = ins.engine
                    ev.sync_info = mybir.SyncInfo(
                        on_wait=extra[i : i + limit], on_update=[]
                    )
                    out.append(ev)
                si.on_wait = keep
                changed = True
            out.append(ins)
        if changed:
            bb.instructions = out


def _build():
    nc = bass.Bass()
    x_ext = nc.declare_dram_parameter("x", [NS, DIM], BF, isOutput=False)
    w18_ext = nc.declare_dram_parameter("w18", [P, DK * HIDDEN], F8, isOutput=False)
    w2b4_ext = nc.declare_dram_parameter("w2b4", [P, OUT_DIM], BF, isOutput=False)
    b1s_ext = nc.declare_dram_parameter("b1s", [P, 1], F32, isOutput=False)
    b2_ext = nc.declare_dram_parameter("b2", [OUT_DIM, 1], F32, isOutput=False)
    b2e_ext = nc.declare_dram_parameter("b2e", [OUT_DIM, 1], F32, isOutput=False)
    y_ext = nc.declare_dram_parameter("y", [NS, OUT_DIM], F32, isOutput=True)

    with _TC(nc) as tc:
        with (
            tc.tile_pool(name="state", bufs=1) as st,
            tc.tile_pool(name="sq", bufs=1) as sqp,
            tc.tile_pool(name="stg", bufs=6) as stg,
            tc.tile_pool(name="xtmp", bufs=1) as xtp,
            tc.tile_pool(name="ps", bufs=8, space="PSUM") as ps,
            tc.tile_pool(name="dram", bufs=2, space="DRAM") as dram,
        ):
            # persistent state
            xr = [st.tile([P, DIM], BF, name=f"xr{i}", tag=f"xr{i}") for i in range(RT)]
            xb8 = [st.tile([P, 2, DIM], F8, name=f"xb8{j}", tag=f"xb8{j}") for j in range(DK)]
            xt8 = st.tile([P, DK, NS], F8, name="xt8", tag="xt8")
            g8 = [st.tile([P, 2, DIM], F8, name=f"g8{k}", tag=f"g8{k}") for k in range(DK // 2)]
            et8 = st.tile([OUT_DIM, NS], F8, name="et8", tag="et8")
            er8 = st.tile([P, RT, OUT_DIM], F8, name="er8", tag="er8")
            er8t = st.tile([P, NS], F8, name="er8t", tag="er8t")
            ptx8 = st.tile([P, DIM], F8, name="ptx8", tag="ptx8")
            a1 = st.tile([P, 512], BF, name="a1", tag="a1")
            w18 = st.tile([P, DK * HIDDEN], F8, name="w18", tag="w18")
            w2b4 = st.tile([P, OUT_DIM], BF, name="w2b4", tag="w2b4")
            b1s = st.tile([P, 1], F32, name="b1s", tag="b1s")
            b2 = st.tile([OUT_DIM, 1], F32, name="b2", tag="b2")
            b2e = st.tile([OUT_DIM, 1], F32, name="b2e", tag="b2e")
            ident8 = st.tile([P, P], F8, name="ident8", tag="ident8")
            identf = st.tile([P, P], F32, name="identf", tag="identf")
            # per-iteration stats, double-buffered across iterations
            n2 = [st.tile([P, RT], F32, name=f"n2{s}", tag=f"n2{s}") for s in range(2)]
            sd = [st.tile([P, RT], F32, name=f"sd{s}", tag=f"sd{s}") for s in range(2)]
            inv = [st.tile([P, RT], F32, name=f"inv{s}", tag=f"inv{s}") for s in range(2)]
            sxv = [st.tile([P, RT], F32, name=f"sxv{s}", tag=f"sxv{s}") for s in range(2)]
            srow = [st.tile([P, RT], F32, name=f"srow{s}", tag=f"srow{s}") for s in range(2)]
            s16 = [st.tile([P, RT], F32, name=f"s16{s}", tag=f"s16{s}") for s in range(2)]
            svc = [st.tile([P, RT], F32, name=f"svc{s}", tag=f"svc{s}") for s in range(2)]

            # loads
            for i in range(RT):
                nc.sync.dma_start(xr[i][:], x_ext[i * P : (i + 1) * P, :])
            nc.sync.dma_start(w18[:], w18_ext[:, :])
            nc.sync.dma_start(w2b4[:], w2b4_ext[:, :])
            nc.sync.dma_start(b1s[:], b1s_ext[:, :])
            nc.sync.dma_start(b2[:], b2_ext[:, :])
            nc.sync.dma_start(b2e[:], b2e_ext[:, :])
            make_identity(nc, identf[:])
            nc.vector.tensor_copy(ident8[:], identf[:])

            def norm_stats(i, s):
                # row norm stats of (raw) block i into stats set s:
                # sq+accum, sqrt on ACT; reciprocal, sxv, fused svc on DVE.
                sq = sqp.tile([P, DIM], BF, name="sq", tag="sq")
                nc.scalar.activation(
                    sq[:], xr[i][:], AF.Square, accum_out=n2[s][:, i : i + 1]
                )
                nc.scalar.sqrt(sd[s][:, i : i + 1], n2[s][:, i : i + 1])
                nc.vector.reciprocal(inv[s][:, i : i + 1], sd[s][:, i : i + 1])
                nc.vector.tensor_scalar_mul(
                    sxv[s][:, i : i + 1], inv[s][:, i : i + 1], SX
                )
                nc.vector.tensor_scalar_mul(
                    svc[s][:, i : i + 1], sd[s][:, i : i + 1], CUV
                )

            def quant_block(i, s):
                # xb8 <- fp8(SX * Xn); issued one block late (QLAG) so the
                # engine queues never head-of-line block on the stats chain.
                dst = xb8[i // 2][:, i % 2, :]
                if i % 2 == 0:
                    nc.vector.tensor_scalar_mul(dst, xr[i][:], sxv[s][:, i : i + 1])
                else:
                    nc.scalar.activation(
                        dst, xr[i][:], AF.Copy, scale=sxv[s][:, i : i + 1]
                    )

            def norm_block(i, s):
                norm_stats(i, s)
                quant_block(i, s)

            def transpose_block(b):
                # xt8[:, k, b*128:(b+1)*128] = fp8(SX*Xn[b-block, :].T)
                # 8 stride-2 transposes packed into one PSUM bank, one
                # multi-dim copy out (engine alternates by block parity).
                pt = ps.tile([P, 2048], F8, name="pstb", tag="ps")
                for k in range(DK):
                    nc.tensor.transpose(
                        pt[:, k * 2 * P : (k + 1) * 2 * P : 2],
                        xb8[b // 2][:, b % 2, k * P : (k + 1) * P],
                        ident8[:],
                    )
                dst = xt8[:, :, b * P : (b + 1) * P]
                if b % 2 == 0:
                    nc.scalar.copy(dst, pt[:, 0:2048:2])
                else:
                    nc.vector.tensor_copy(dst, pt[:, 0:2048:2])

            def phase_gram(ms, h, arin, drain_rr, row0=0):
                # partial (SX Xn).T @ (SX Xn) over row tiles for m-blocks `ms`,
                # column half h; drain scaled to -8*G fp8 into arin rows
                # (m-row0)*128.
                for m in ms:
                    pg = ps.tile([P, 512], F32, name="ps", tag="ps")
                    for j in range(DK):
                        nc.tensor.matmul(
                            pg[:],
                            xb8[j][:, :, m * P : (m + 1) * P],
                            xb8[j][:, :, h * 512 : (h + 1) * 512],
                            start=(j == 0),
                            stop=(j == DK - 1),
                            perf_mode=DR,
                        )
                    gs = stg.tile([P, 512], F8, name="gs", tag="gs")
                    if drain_rr.pop(0) == "a":
                        nc.scalar.mul(gs[:], pg[:], GRAM_DRAIN)
                    else:
                        nc.vector.tensor_scalar_mul(gs[:], pg[:], GRAM_DRAIN)
                    nc.sync.dma_start(arin[(m - row0) * P : (m - row0 + 1) * P, :], gs[:])

            def phase_mlp_q(q, pa, et_dst, bias, act_fn):
                # one col-tiled MLP1 chain (M=16 in col group q, non-DR fp8)
                # + its relu, row-tiled MLP2 matmul (K=16 in row group q) and
                # output activation. Chains for different q are independent
                # and run concurrently in the array.
                for j in range(DK):
                    nc.tensor.matmul(
                        pa[32 * q : 32 * q + HIDDEN, :],
                        w18[:, j * HIDDEN : (j + 1) * HIDDEN],
                        xt8[:, j, q * 512 : (q + 1) * 512],
                        start=(j == 0),
                        stop=(j == DK - 1),
                        tile_position=(0, 32 * q),
                    )
                nc.scalar.activation(
                    a1[32 * q : 32 * q + HIDDEN, :],
                    pa[32 * q : 32 * q + HIDDEN, :],
                    AF.Relu,
                    bias=b1s[32 * q : 32 * q + HIDDEN, :],
                    scale=MLP1_SCALE,
                )
                pb = ps.tile([OUT_DIM, 512], F32, name="ps2", tag="ps")
                nc.tensor.matmul(
                    pb[:],
                    w2b4[32 * q : 32 * q + HIDDEN, :],
                    a1[32 * q : 32 * q + HIDDEN, :],
                    tile_position=(32 * q, 0),
                )
                sl = slice(q * 512, (q + 1) * 512)
                nc.scalar.activation(et_dst[:, sl], pb[:], act_fn, bias=bias[:])

            def phase_et_transpose(s):
                # transpose et8 (4E) to rows (stride-2 fp8); srow = sum(4E)
                pts = []
                for j2 in range(2):
                    pt = ps.tile([P, 1024], F8, name="ps8", tag="ps")
                    for q in range(8):
                        i = 8 * j2 + q
                        nc.tensor.transpose(
                            pt[:, q * 2 * OUT_DIM : (q + 1) * 2 * OUT_DIM : 2],
                            et8[:, i * P : (i + 1) * P],
                            ident8[:OUT_DIM, :OUT_DIM],
                        )
                    nc.vector.tensor_reduce(
                        srow[s][:, 8 * j2 : 8 * j2 + 8],
                        pt[:].rearrange("p (i o t) -> p i o t", o=OUT_DIM, t=2)[:, :, :, 0],
                        mybir.AxisListType.X,
                        ALU.add,
                    )
                    pts.append(pt)
                return pts

            def phase_p(pts, s):
                # er8[:, i, :] = fp8(SP * P-rows) = pt * s16 (split ACT/DVE)
                nc.vector.reciprocal(s16[s][:], srow[s][:])
                nc.vector.tensor_scalar_mul(s16[s][:], s16[s][:], SP)
                for j2 in range(2):
                    for q in range(8):
                        i = 8 * j2 + q
                        src = pts[j2][:, q * 2 * OUT_DIM : (q + 1) * 2 * OUT_DIM : 2]
                        if i % 2 == 0:
                            nc.vector.tensor_scalar_mul(
                                er8[:, i, :], src, s16[s][:, i : i + 1]
                            )
                        else:
                            nc.scalar.mul(
                                er8[:, i, :], src, s16[s][:, i : i + 1]
                            )

            def phase_er8t():
                # er8t = (16P).T via PE transposes of er8 rows; mirrored to
                # partitions 64-127 with one SBUF->SBUF DMA for U row-tiling.
                for j2 in range(2):
                    pt = ps.tile([OUT_DIM, 2048], F8, name="pse", tag="ps")
                    for q in range(8):
                        i = 8 * j2 + q
                        nc.tensor.transpose(
                            pt[:, q * 2 * P : (q + 1) * 2 * P : 2],
                            er8[:, i, :],
                            ident8[:],
                        )
                    if j2 == 0:
                        nc.scalar.copy(
                            er8t[:OUT_DIM, j2 * 1024 : (j2 + 1) * 1024], pt[:, 0:2048:2]
                        )
                    else:
                        nc.vector.tensor_copy(
                            er8t[:OUT_DIM, j2 * 1024 : (j2 + 1) * 1024], pt[:, 0:2048:2]
                        )
                nc.sync.dma_start(er8t[OUT_DIM : 2 * OUT_DIM, :], er8t[:OUT_DIM, :])

            def phase_ptx(arin):
                # partial (SP*P).T @ (SX*Xn) -> fp8(SG*PtX) wire
                for h in range(2):
                    pp = ps.tile([OUT_DIM, 512], F32, name="ps", tag="ps")
                    for j in range(DK):
                        nc.tensor.matmul(
                            pp[:],
                            er8[:, 2 * j : 2 * j + 2, :],
                            xb8[j][:, :, h * 512 : (h + 1) * 512],
                            start=(j == 0),
                            stop=(j == DK - 1),
                            perf_mode=DR,
                        )
                    pps = stg.tile([OUT_DIM, 512], F8, name="pps", tag="gs")
                    nc.scalar.mul(pps[:], pp[:], PTX_DRAIN)
                    nc.sync.dma_start(arin[:, h * 512 : (h + 1) * 512], pps[:])

            def phase_bl():
                # bottom-left of g8 = transpose(top-right): g8 cols 0:512 for
                # k-blocks 4..7 from g8 cols 512:1024 of k-blocks 0..3.
                for b in range(4):
                    pt = ps.tile([P, 1024], F8, name="ps8", tag="ps")
                    for a in range(4):
                        nc.tensor.transpose(
                            pt[:, a * 2 * P : (a + 1) * 2 * P : 2],
                            g8[a // 2][:, a % 2, 512 + b * P : 512 + (b + 1) * P],
                            ident8[:],
                        )
                    if b % 2 == 0:
                        nc.scalar.copy(g8[2 + b // 2][:, b % 2, 0:512], pt[:, 0:1024:2])
                    else:
                        nc.vector.tensor_copy(
                            g8[2 + b // 2][:, b % 2, 0:512], pt[:, 0:1024:2]
                        )

            def phase_uv(h, s, tail=None, ulag=0):
                # per block: fused PSUM chain 128*(P@PtX - Xn@G) cols h via
                # 4 DR matmuls + a U matmul; U matmuls run pairwise
                # concurrent via row tiles (0,0)/(64,0) using the er8t/ptx8
                # partition mirrors. Then one STT: xsl += svc * psum.
                # ulag defers each pair's U+STT by that many pairs so the
                # U matmuls never wait on the PtX AllReduce landing.
                pus = {}
                def u_and_stt(pair):
                    for ii, tp in ((2 * pair, 0), (2 * pair + 1, OUT_DIM)):
                        nc.tensor.matmul(
                            pus[ii][:],
                            er8t[tp : tp + OUT_DIM, ii * P : (ii + 1) * P],
                            ptx8[tp : tp + OUT_DIM, h * 512 : (h + 1) * 512],
                            start=False,
                            stop=True,
                            tile_position=(tp, 0),
                        )
                    for ii in (2 * pair, 2 * pair + 1):
                        xsl = xr[ii][:, h * 512 : (h + 1) * 512]
                        nc.vector.scalar_tensor_tensor(
                            xsl, pus.pop(ii)[:], svc[s][:, ii : ii + 1], xsl,
                            ALU.mult, ALU.add,
                        )
                        if tail is not None:
                            tail(ii)
                for i in range(RT):
                    pu = ps.tile([P, 512], F32, name="ps", tag="ps")
                    pus[i] = pu
                    for kk in range(DK // 2):
                        nc.tensor.matmul(
                            pu[:],
                            xt8[:, 2 * kk : 2 * kk + 2, i * P : (i + 1) * P],
                            g8[kk][:, :, h * 512 : (h + 1) * 512],
                            start=(kk == 0),
                            stop=False,
                            perf_mode=DR,
                        )
                    if i % 2 == 1:
                        pair = (i - 1) // 2
                        if pair >= ulag:
                            u_and_stt(pair - ulag)
                for pair in range(RT // 2 - ulag, RT // 2):
                    u_and_stt(pair)

            rg = [list(range(CORES))]
            # initial norm pass with iter-0 transposes interleaved (PE is
            # otherwise idle while ACT/DVE normalize).
            for i in range(RT):
                norm_block(i, 0)
                if i >= TLAG:
                    transpose_block(i - TLAG)
            for b in range(RT - TLAG, RT):
                transpose_block(b)

            for it in range(DEPTH):
                s = it % 2
                arin_tr = dram.tile([512, 512], F8, name="arin_tr", tag="arin_tr")
                arout_tr = dram.tile([512, 512], F8, name="arout_tr", tag="arout_tr", addr_space="Shared")
                arin_br = dram.tile([512, 512], F8, name="arin_br", tag="arin_br")
                arout_br = dram.tile([512, 512], F8, name="arout_br", tag="arout_br", addr_space="Shared")
                arin_tl = dram.tile([512, 512], F8, name="arin_tl", tag="arin_tl")
                arout_tl = dram.tile([512, 512], F8, name="arout_tl", tag="arout_tl", addr_space="Shared")
                arin_p = dram.tile([OUT_DIM, DIM], F8, name="arin_p", tag="arin_p")
                arout_p = dram.tile([OUT_DIM, DIM], F8, name="arout_p", tag="arout_p", addr_space="Shared")

                drains = list("avavavav")
                phase_gram(range(DK // 2), 1, arin_tr, drains[:4], row0=0)
                nc.gpsimd.collective_compute(
                    "AllReduce", ALU.add,
                    ins=[arin_tr.opt()], outs=[arout_tr.opt()], replica_groups=rg,
                )
                phase_gram(range(DK // 2, DK), 1, arin_br, drains[4:], row0=DK // 2)
                nc.gpsimd.collective_compute(
                    "AllReduce", ALU.add,
                    ins=[arin_br.opt()], outs=[arout_br.opt()], replica_groups=rg,
                )
                # tl gram drains now; its AllReduce is emitted AFTER the PtX
                # one (uv h=1's U matmuls need p mid-loop; tl is only needed
                # at uv h=0). The transposes sit in the AllReduce latency
                # shadow (iter 0's ran interleaved with the initial norm).
                phase_gram(range(DK // 2), 0, arin_tl, list("avav"))
                if it > 0:
                    for b in range(RT):
                        transpose_block(b)
                pa = ps.tile([P, 512], F32, name="ps", tag="ps")
                for q in range(4):
                    phase_mlp_q(q, pa, et8, b2e, AF.Exp)
                pts = phase_et_transpose(s)
                phase_p(pts, s)
                phase_ptx(arin_p)
                nc.gpsimd.collective_compute(
                    "AllReduce", ALU.add,
                    ins=[arin_p.opt()], outs=[arout_p.opt()], replica_groups=rg,
                )
                nc.gpsimd.collective_compute(
                    "AllReduce", ALU.add,
                    ins=[arin_tl.opt()], outs=[arout_tl.opt()], replica_groups=rg,
                )
                phase_er8t()
                # land AllReduce results
                for k in range(DK // 2):
                    nc.sync.dma_start(
                        g8[k // 2][:, k % 2, 512:1024],
                        arout_tr[k * P : (k + 1) * P, :],
                    )
                for k in range(DK // 2, DK):
                    nc.sync.dma_start(
                        g8[k // 2][:, k % 2, 512:1024],
                        arout_br[(k - DK // 2) * P : (k - DK // 2 + 1) * P, :],
                    )
                for k in range(DK // 2):
                    nc.sync.dma_start(
                        g8[k // 2][:, k % 2, 0:512],
                        arout_tl[k * P : (k + 1) * P, :],
                    )
                nc.sync.dma_start(ptx8[:OUT_DIM, :], arout_p[:, :])
                nc.sync.dma_start(ptx8[OUT_DIM : 2 * OUT_DIM, :], arout_p[:, :])
                phase_bl()
                phase_uv(1, s, ulag=3)
                so = (it + 1) % 2
                if it < DEPTH - 1:
                    def tail(i, it=it, s=s, so=so):
                        norm_stats(i, so)
                        if i >= 1:
                            quant_block(i - 1, so)
                    phase_uv(0, s, tail=tail)
                    quant_block(RT - 1, so)
                else:
                    # fb8 = fp8(SX * X_6) = fp8(S * SX*inv) into xb8 tiles
                    # (X_6 = S_6 * inv_5; xb8 is dead after this iter's
                    # gram/ptx so the tiles are recycled for the final MLP)
                    def fquant(i, s=s, so=so):
                        dst = xb8[i // 2][:, i % 2, :]
                        if i % 2 == 0:
                            nc.vector.tensor_scalar_mul(
                                dst, xr[i][:], sxv[so][:, i : i + 1]
                            )
                        else:
                            nc.scalar.activation(
                                dst, xr[i][:], AF.Copy,
                                scale=sxv[so][:, i : i + 1],
                            )
                    def tail(i, it=it, s=s, so=so):
                        nc.vector.tensor_scalar_mul(
                            sxv[so][:, i : i + 1], inv[s][:, i : i + 1], SX
                        )
                        if i >= 1:
                            fquant(i - 1)
                    phase_uv(0, s, tail=tail)
                    fquant(RT - 1)

            # final MLP in fp8 on the final state's transposes
            yt = xtp.tile([OUT_DIM, NS], F32, name="yt", tag="yt")
            pa = ps.tile([P, 512], F32, name="ps", tag="ps")
            for q in range(4):
                for b in range(4 * q, 4 * q + 4):
                    transpose_block(b)
                phase_mlp_q(q, pa, yt, b2, AF.Identity)
            # transpose Y.T -> rows and store
            yr = sqp.tile([P, RT, OUT_DIM], F32, name="yr", tag="sq")
            for j2 in range(2):
                pt = ps.tile([P, 512], F32, name="ps", tag="ps")
                for q in range(8):
                    i = 8 * j2 + q
                    nc.tensor.transpose(
                        pt[:, q * OUT_DIM : (q + 1) * OUT_DIM],
                        yt[:, i * P : (i + 1) * P].bitcast(F32),
                        identf[:OUT_DIM, :OUT_DIM],
                    )
                nc.vector.tensor_copy(yr[:, 8 * j2 : 8 * j2 + 8, :], pt[:])
            nc.sync.dma_start(
                y_ext.rearrange("(i p) o -> p i o", p=P), yr[:, :, :]
            )

    _split_waits(nc)
    return nc


_NC = None


def _get_nc():
    global _NC
    if _NC is None:
        _NC = _build()
    return _NC


def _q8(x):
    return np.clip(x, -240.0, 240.0).astype(ml_dtypes.float8_e4m3)


def _in_maps(X, W1, b1, W2, b2):
    X = np.asarray(X, dtype=np.float32)
    W1 = np.asarray(W1, dtype=np.float32)
    b1 = np.asarray(b1, dtype=np.float32).reshape(HIDDEN)
    W2 = np.asarray(W2, dtype=np.float32)
    b2c = np.asarray(b2, dtype=np.float32).reshape(OUT_DIM, 1)
    b2e = b2c + np.float32(math.log(SE))
    # w18: fp8(SW1*W1) packed [128, j, h] flat (non-interleaved; MLP1 runs
    # non-DR col-tiled)
    w18 = np.zeros((P, DK * HIDDEN), np.float32)
    for j in range(DK):
        w18[:, j * HIDDEN : (j + 1) * HIDDEN] = W1[j * P : (j + 1) * P, :] * SW1
    w18 = _q8(w18)
    # w2b4/b1s: replicated into partition groups 32q..32q+15 for the
    # col/row-tiled MLP
    w2b4 = np.zeros((P, OUT_DIM), np.float32)
    b1s = np.zeros((P, 1), np.float32)
    for q in range(4):
        w2b4[32 * q : 32 * q + HIDDEN, :] = W2
        b1s[32 * q : 32 * q + HIDDEN, 0] = b1
    w2b4 = w2b4.astype(ml_dtypes.bfloat16)
    Xb = X.astype(ml_dtypes.bfloat16)
    return [
        {
            "x": np.ascontiguousarray(Xb[c * NS : (c + 1) * NS]),
            "w18": w18,
            "w2b4": w2b4,
            "b1s": b1s,
            "b2": b2c,
            "b2e": b2e,
        }
        for c in range(CORES)
    ]


def run(X, W1, b1, W2, b2, **kwargs):
    nc = _get_nc()
    res = run_bass_kernel_spmd(nc, _in_maps(X, W1, b1, W2, b2), list(range(CORES)), **kwargs)
    out = np.concatenate([res.results[c]["y"] for c in range(CORES)], axis=0)
    return out, res


def kernel(X, W1, b1, W2, b2):
    out, _ = run(X, W1, b1, W2, b2)
    return out


# revision 28
# speedup vs baseline: 1.0829x; 1.0297x over previous
"""CKAFormer Trainium2 kernel, fp8 edition.

6 iterations of
    Xn = X / ||X||_row;  P = softmax(relu(Xn@W1+b1)@W2+b2)
    X  = Xn + g*P@(P.T@Xn) - g*Xn@(Xn.T@Xn)
then a final MLP. Row-sharded over 8 NeuronCores.

Speed scheme vs the bf16 baseline:
- State is kept per-row-SCALED (S = nrm*X): the row normalization of the
  leading term cancels. True X is recovered by one in-place scale pass in
  the last iteration only.
- U and V updates accumulate into ONE PSUM bank: with g8 = -8*G,
  er8 = 16*P, ptx8 = 8*PtX and xt8 = 16*Xn^T, both terms come out as
  128*(P@PtX - Xn@G), so a single scalar_tensor_tensor with per-row
  scalar sd*GAMMA/128 applies the whole update.
- Big in-loop matmuls (Gram, V, PtX) run fp8e4m3 DoubleRow (2 k-blocks
  per pass). The U matmuls (K=64) run pairwise-concurrent via
  tile_position row tiling (er8t/ptx8 mirrored to partitions 64-127).
- MLP1 (M=16) runs as 4 concurrent col-tiled chains (col groups 0..3,
  non-DR fp8); MLP2 (K=16) as 4 concurrent row-tiled matmuls. w2b/b1
  are host-replicated across the 4 partition groups.
- Xn^T transposes are produced per-block in the uv(h=0) tail (lag 2)
  right after each block's update+renorm+quantize, so they fill PE slack
  during the DVE-bound uv phase instead of forming a serial phase.
  Iteration 0 interleaves them into the initial norm pass.
- G is AllReduced in fp8 as top-right + bottom-right + top-left [512,512]
  chunks; the bottom-left quadrant is reconstructed locally as
  transpose(top-right). PtX is a fourth fp8 AllReduce. Update error
  enters X only through GAMMA=1e-4.
- Element-wise work is split across DVE and ACT only (GpSimd ucode
  tensor ops are ~10x slower and cannot touch PSUM).
The final MLP also runs fp8, reusing w18 and the tail transposes of the
last iteration's output.
"""

import sys

sys.path.insert(0, "/opt/trn_rl_repo")

import math

import ml_dtypes
import numpy as np

import concourse.bass as bass
import concourse.mybir as mybir
import concourse.tile as tile
from concourse.bass_utils import run_bass_kernel_spmd
from concourse.masks import make_identity
from concourse.vector_clock import ScopedClock

DEPTH = 6
GAMMA = 1e-4
DIM = 1024
HIDDEN = 16
OUT_DIM = 64
N = 16384
CORES = 8

NS = N // CORES        # rows per core = 2048
RT = NS // 128         # row tiles = 16
DK = DIM // 128        # dim k-tiles = 8
P = 128

F32 = mybir.dt.float32
F32R = mybir.dt.float32r
BF = mybir.dt.bfloat16
F8 = mybir.dt.float8e4
AF = mybir.ActivationFunctionType
ALU = mybir.AluOpType
DR = mybir.MatmulPerfMode.DoubleRow

SX = 16.0    # xb8/xt8 = SX * Xn
SW1 = 32.0   # w18 = SW1 * W1
SP = 16.0    # er8/er8t = SP * P
SE = 4.0     # et8 = SE * E
SG = 8.0     # g8 wire = -SG * G ;  ptx8 wire = SG * PtX

GRAM_DRAIN = -SG / (SX * SX)           # psum(SX^2 G) -> -8*G
PTX_DRAIN = SG / (SP * SX)             # psum(SP*SX*PtX) -> 8*PtX
MLP1_SCALE = 1.0 / (SX * SW1)          # psum -> Xn@W1
CUV = GAMMA / (SX * SG)                # fused STT: svc = sd*GAMMA/128

TLAG = 6  # tail transposes run this many blocks behind the uv update

# this container's walrus only accepts one sync-wait slot per engine
# instruction; hoist excess waits onto preceding EventSemaphore carriers.
_MAX_WAITS = 1


class _TC(tile.TileContext):
    def _drain_and_barrier(self, tick_clock, wait_clock):
        drain_inst = self.nc.sync.drain()
        wait_clock.add_sem_waits(
            drain_inst.ins, ScopedClock({None: tick_clock.global_clock})
        )
        si = drain_inst.ins.sync_info
        w = list(si.on_wait) if si and si.on_wait else []
        if len(w) > 1:
            si.on_wait = w[:1]
            for i in range(1, len(w)):
                c = self.nc.sync.drain()
                c.ins.sync_info = mybir.SyncInfo(on_wait=[w[i]], on_update=[])
        self.nc.all_engine_barrier()
        assert self.sems is not None
        popped = self.nc._tile_sem_poison_stack.pop()
        assert popped is self._sem_poison
        self.nc.clear_and_free_semaphores(list(self.sems.allocated().values()))
        self.nc.all_engine_barrier()


def _split_waits(nc, limit=_MAX_WAITS):
    """Hoist excess sem waits onto EventSemaphore carriers inserted just
    before the over-limit instruction (per-engine program order preserves the
    gating; waits are a conjunction so splitting is sound)."""
    nid = 0
    for bb in nc.main_func.blocks:
        out = []
        changed = False
        for ins in bb.instructions:
            si = ins.sync_info
            w = list(si.on_wait) if si and si.on_wait else []
            if len(w) > limit:
                extra, keep = w[:-limit], w[-limit:]
                for i in range(0, len(extra), limit):
                    ev = mybir.InstEventSemaphore(name=f"wsplit_{nid}", ins=[], outs=[])
                    nid += 1
                    ev.engine = ins.engine
                    ev.sync_info = mybir.SyncInfo(
                        on_wait=extra[i : i + limit], on_update=[]
                    )
                    out.append(ev)
                si.on_wait = keep
                changed = True
            out.append(ins)
        if changed:
            bb.instructions = out


def _build():
    nc = bass.Bass()
    x_ext = nc.declare_dram_parameter("x", [NS, DIM], BF, isOutput=False)
    w18_ext = nc.declare_dram_parameter("w18", [P, DK * HIDDEN], F8, isOutput=False)
    w2b4_ext = nc.declare_dram_parameter("w2b4", [P, OUT_DIM], BF, isOutput=False)
    b1s_ext = nc.declare_dram_parameter("b1s", [P, 1], F32, isOutput=False)
    b2_ext = nc.declare_dram_parameter("b2", [OUT_DIM, 1], F32, isOutput=False)
    b2e_ext = nc.declare_dram_parameter("b2e", [OUT_DIM, 1], F32, isOutput=False)
    y_ext = nc.declare_dram_parameter("y", [NS, OUT_DIM], F32, isOutput=True)

    with _TC(nc) as tc:
        with (
            tc.tile_pool(name="state", bufs=1) as st,
            tc.tile_pool(name="sq", bufs=1) as sqp,
            tc.tile_pool(name="stg", bufs=6) as stg,
            tc.tile_pool(name="xtmp", bufs=1) as xtp,
            tc.tile_pool(name="ps", bufs=8, space="PSUM") as ps,
            tc.tile_pool(name="dram", bufs=2, space="DRAM") as dram,
        ):
            # persistent state
            xr = [st.tile([P, DIM], BF, name=f"xr{i}", tag=f"xr{i}") for i in range(RT)]
            xb8 = [st.tile([P, 2, DIM], F8, name=f"xb8{j}", tag=f"xb8{j}") for j in range(DK)]
            xt8 = st.tile([P, DK, NS], F8, name="xt8", tag="xt8")
            g8 = [st.tile([P, 2, DIM], F8, name=f"g8{k}", tag=f"g8{k}") for k in range(DK // 2)]
            et8 = st.tile([OUT_DIM, NS], F8, name="et8", tag="et8")
            er8 = st.tile([P, RT, OUT_DIM], F8, name="er8", tag="er8")
            er8t = st.tile([P, NS], F8, name="er8t", tag="er8t")
            ptx8 = st.tile([P, DIM], F8, name="ptx8", tag="ptx8")
            a1 = st.tile([P, 512], BF, name="a1", tag="a1")
            w18 = st.tile([P, DK * HIDDEN], F8, name="w18", tag="w18")
            w2b4 = st.tile([P, OUT_DIM], BF, name="w2b4", tag="w2b4")
            b1s = st.tile([P, 1], F32, name="b1s", tag="b1s")
            b2 = st.tile([OUT_DIM, 1], F32, name="b2", tag="b2")
            b2e = st.tile([OUT_DIM, 1], F32, name="b2e", tag="b2e")
            ident8 = st.tile([P, P], F8, name="ident8", tag="ident8")
            identf = st.tile([P, P], F32, name="identf", tag="identf")
            # per-iteration stats, double-buffered across iterations
            n2 = [st.tile([P, RT], F32, name=f"n2{s}", tag=f"n2{s}") for s in range(2)]
            sd = [st.tile([P, RT], F32, name=f"sd{s}", tag=f"sd{s}") for s in range(2)]
            inv = [st.tile([P, RT], F32, name=f"inv{s}", tag=f"inv{s}") for s in range(2)]
            sxv = [st.tile([P, RT], F32, name=f"sxv{s}", tag=f"sxv{s}") for s in range(2)]
            srow = [st.tile([P, RT], F32, name=f"srow{s}", tag=f"srow{s}") for s in range(2)]
            s16 = [st.tile([P, RT], F32, name=f"s16{s}", tag=f"s16{s}") for s in range(2)]
            svc = [st.tile([P, RT], F32, name=f"svc{s}", tag=f"svc{s}") for s in range(2)]

            # loads
            for i in range(RT):
                nc.sync.dma_start(xr[i][:], x_ext[i * P : (i + 1) * P, :])
            nc.sync.dma_start(w18[:], w18_ext[:, :])
            nc.sync.dma_start(w2b4[:], w2b4_ext[:, :])
            nc.sync.dma_start(b1s[:], b1s_ext[:, :])
            nc.sync.dma_start(b2[:], b2_ext[:, :])
            nc.sync.dma_start(b2e[:], b2e_ext[:, :])
            make_identity(nc, identf[:])
            nc.vector.tensor_copy(ident8[:], identf[:])

            def norm_stats(i, s):
                # row norm stats of (raw) block i into stats set s:
                # sq+accum, sqrt on ACT; reciprocal, sxv, fused svc on DVE.
                sq = sqp.tile([P, DIM], BF, name="sq", tag="sq")
                nc.scalar.activation(
                    sq[:], xr[i][:], AF.Square, accum_out=n2[s][:, i : i + 1]
                )
                nc.scalar.sqrt(sd[s][:, i : i + 1], n2[s][:, i : i + 1])
                nc.vector.reciprocal(inv[s][:, i : i + 1], sd[s][:, i : i + 1])
                nc.vector.tensor_scalar_mul(
                    sxv[s][:, i : i + 1], inv[s][:, i : i + 1], SX
                )
                nc.vector.tensor_scalar_mul(
                    svc[s][:, i : i + 1], sd[s][:, i : i + 1], CUV
                )

            def quant_block(i, s):
                # xb8 <- fp8(SX * Xn); issued one block late (QLAG) so the
                # engine queues never head-of-line block on the stats chain.
                dst = xb8[i // 2][:, i % 2, :]
                if i % 2 == 0:
                    nc.vector.tensor_scalar_mul(dst, xr[i][:], sxv[s][:, i : i + 1])
                else:
                    nc.scalar.activation(
                        dst, xr[i][:], AF.Copy, scale=sxv[s][:, i : i + 1]
                    )

            def norm_block(i, s):
                norm_stats(i, s)
                quant_block(i, s)

            def transpose_block(b):
                # xt8[:, k, b*128:(b+1)*128] = fp8(SX*Xn[b-block, :].T)
                # 8 stride-2 transposes packed into one PSUM bank, one
                # multi-dim copy out (engine alternates by block parity).
                pt = ps.tile([P, 2048], F8, name="pstb", tag="ps")
                for k in range(DK):
                    nc.tensor.transpose(
                        pt[:, k * 2 * P : (k + 1) * 2 * P : 2],
                        xb8[b // 2][:, b % 2, k * P : (k + 1) * P],
                        ident8[:],
                    )
                dst = xt8[:, :, b * P : (b + 1) * P]
                if b % 2 == 0:
                    nc.scalar.copy(dst, pt[:, 0:2048:2])
                else:
                    nc.vector.tensor_copy(dst, pt[:, 0:2048:2])

            def phase_gram(ms, h, arin, drain_rr, row0=0):
                # partial (SX Xn).T @ (SX Xn) over row tiles for m-blocks `ms`,
                # column half h; drain scaled to -8*G fp8 into arin rows
                # (m-row0)*128.
                for m in ms:
                    pg = ps.tile([P, 512], F32, name="ps", tag="ps")
                    for j in range(DK):
                        nc.tensor.matmul(
                            pg[:],
                            xb8[j][:, :, m * P : (m + 1) * P],
                            xb8[j][:, :, h * 512 : (h + 1) * 512],
                            start=(j == 0),
                            stop=(j == DK - 1),
                            perf_mode=DR,
                        )
                    gs = stg.tile([P, 512], F8, name="gs", tag="gs")
                    if drain_rr.pop(0) == "a":
                        nc.scalar.mul(gs[:], pg[:], GRAM_DRAIN)
                    else:
                        nc.vector.tensor_scalar_mul(gs[:], pg[:], GRAM_DRAIN)
                    nc.sync.dma_start(arin[(m - row0) * P : (m - row0 + 1) * P, :], gs[:])

            def phase_mlp(et_dst, bias, act_fn):
                # 4 col-tiled MLP1 chains (M=16 in col groups 0..3, non-DR
                # fp8) issued j-major so the four chains run concurrently in
                # the array, + 4 row-tiled MLP2 matmuls (K=16 in row groups).
                pa = ps.tile([P, 512], F32, name="ps", tag="ps")
                for j in range(DK):
                    for q in range(4):
                        nc.tensor.matmul(
                            pa[32 * q : 32 * q + HIDDEN, :],
                            w18[:, j * HIDDEN : (j + 1) * HIDDEN],
                            xt8[:, j, q * 512 : (q + 1) * 512],
                            start=(j == 0),
                            stop=(j == DK - 1),
                            tile_position=(0, 32 * q),
                        )
                for q in range(4):
                    nc.scalar.activation(
                        a1[32 * q : 32 * q + HIDDEN, :],
                        pa[32 * q : 32 * q + HIDDEN, :],
                        AF.Relu,
                        bias=b1s[32 * q : 32 * q + HIDDEN, :],
                        scale=MLP1_SCALE,
                    )
                pbs = []
                for q in range(4):
                    pb = ps.tile([OUT_DIM, 512], F32, name="ps2", tag="ps")
                    nc.tensor.matmul(
                        pb[:],
                        w2b4[32 * q : 32 * q + HIDDEN, :],
                        a1[32 * q : 32 * q + HIDDEN, :],
                        tile_position=(32 * q, 0),
                    )
                    pbs.append(pb)
                for q in range(4):
                    sl = slice(q * 512, (q + 1) * 512)
                    nc.scalar.activation(
                        et_dst[:, sl], pbs[q][:], act_fn, bias=bias[:]
                    )

            def phase_et_transpose(s):
                # transpose et8 (4E) to rows (stride-2 fp8); srow = sum(4E)
                pts = []
                for j2 in range(2):
                    pt = ps.tile([P, 1024], F8, name="ps8", tag="ps")
                    for q in range(8):
                        i = 8 * j2 + q
                        nc.tensor.transpose(
                            pt[:, q * 2 * OUT_DIM : (q + 1) * 2 * OUT_DIM : 2],
                            et8[:, i * P : (i + 1) * P],
                            ident8[:OUT_DIM, :OUT_DIM],
                        )
                    nc.vector.tensor_reduce(
                        srow[s][:, 8 * j2 : 8 * j2 + 8],
                        pt[:].rearrange("p (i o t) -> p i o t", o=OUT_DIM, t=2)[:, :, :, 0],
                        mybir.AxisListType.X,
                        ALU.add,
                    )
                    pts.append(pt)
                return pts

            def phase_p(pts, s):
                # er8[:, i, :] = fp8(SP * P-rows) = pt * s16 (split ACT/DVE)
                nc.vector.reciprocal(s16[s][:], srow[s][:])
                nc.vector.tensor_scalar_mul(s16[s][:], s16[s][:], SP)
                for j2 in range(2):
                    for q in range(8):
                        i = 8 * j2 + q
                        src = pts[j2][:, q * 2 * OUT_DIM : (q + 1) * 2 * OUT_DIM : 2]
                        if i % 2 == 0:
                            nc.vector.tensor_scalar_mul(
                                er8[:, i, :], src, s16[s][:, i : i + 1]
                            )
                        else:
                            nc.scalar.mul(
                                er8[:, i, :], src, s16[s][:, i : i + 1]
                            )

            def phase_er8t():
                # er8t = (16P).T via PE transposes of er8 rows; mirrored to
                # partitions 64-127 with one SBUF->SBUF DMA for U row-tiling.
                for j2 in range(2):
                    pt = ps.tile([OUT_DIM, 2048], F8, name="pse", tag="ps")
                    for q in range(8):
                        i = 8 * j2 + q
                        nc.tensor.transpose(
                            pt[:, q * 2 * P : (q + 1) * 2 * P : 2],
                            er8[:, i, :],
                            ident8[:],
                        )
                    if j2 == 0:
                        nc.scalar.copy(
                            er8t[:OUT_DIM, j2 * 1024 : (j2 + 1) * 1024], pt[:, 0:2048:2]
                        )
                    else:
                        nc.vector.tensor_copy(
                            er8t[:OUT_DIM, j2 * 1024 : (j2 + 1) * 1024], pt[:, 0:2048:2]
                        )
                nc.sync.dma_start(er8t[OUT_DIM : 2 * OUT_DIM, :], er8t[:OUT_DIM, :])

            def phase_ptx(arin):
                # partial (SP*P).T @ (SX*Xn) -> fp8(SG*PtX) wire
                for h in range(2):
                    pp = ps.tile([OUT_DIM, 512], F32, name="ps", tag="ps")
                    for j in range(DK):
                        nc.tensor.matmul(
                            pp[:],
                            er8[:, 2 * j : 2 * j + 2, :],
                            xb8[j][:, :, h * 512 : (h + 1) * 512],
                            start=(j == 0),
                            stop=(j == DK - 1),
                            perf_mode=DR,
                        )
                    pps = stg.tile([OUT_DIM, 512], F8, name="pps", tag="gs")
                    nc.scalar.mul(pps[:], pp[:], PTX_DRAIN)
                    nc.sync.dma_start(arin[:, h * 512 : (h + 1) * 512], pps[:])

            def phase_bl():
                # bottom-left of g8 = transpose(top-right): g8 cols 0:512 for
                # k-blocks 4..7 from g8 cols 512:1024 of k-blocks 0..3.
                for b in range(4):
                    pt = ps.tile([P, 1024], F8, name="ps8", tag="ps")
                    for a in range(4):
                        nc.tensor.transpose(
                            pt[:, a * 2 * P : (a + 1) * 2 * P : 2],
                            g8[a // 2][:, a % 2, 512 + b * P : 512 + (b + 1) * P],
                            ident8[:],
                        )
                    if b % 2 == 0:
                        nc.scalar.copy(g8[2 + b // 2][:, b % 2, 0:512], pt[:, 0:1024:2])
                    else:
                        nc.vector.tensor_copy(
                            g8[2 + b // 2][:, b % 2, 0:512], pt[:, 0:1024:2]
                        )

            def phase_uv(h, s, tail=None, ulag=0):
                # per block: fused PSUM chain 128*(P@PtX - Xn@G) cols h via
                # 4 DR matmuls + a U matmul; U matmuls run pairwise
                # concurrent via row tiles (0,0)/(64,0) using the er8t/ptx8
                # partition mirrors. Then one STT: xsl += svc * psum.
                # ulag defers each pair's U+STT by that many pairs so the
                # U matmuls never wait on the PtX AllReduce landing.
                pus = {}
                def u_and_stt(pair):
                    for ii, tp in ((2 * pair, 0), (2 * pair + 1, OUT_DIM)):
                        nc.tensor.matmul(
                            pus[ii][:],
                            er8t[tp : tp + OUT_DIM, ii * P : (ii + 1) * P],
                            ptx8[tp : tp + OUT_DIM, h * 512 : (h + 1) * 512],
                            start=False,
                            stop=True,
                            tile_position=(tp, 0),
                        )
                    for ii in (2 * pair, 2 * pair + 1):
                        xsl = xr[ii][:, h * 512 : (h + 1) * 512]
                        nc.vector.scalar_tensor_tensor(
                            xsl, pus.pop(ii)[:], svc[s][:, ii : ii + 1], xsl,
                            ALU.mult, ALU.add,
                        )
                        if tail is not None:
                            tail(ii)
                for i in range(RT):
                    pu = ps.tile([P, 512], F32, name="ps", tag="ps")
                    pus[i] = pu
                    for kk in range(DK // 2):
                        nc.tensor.matmul(
                            pu[:],
                            xt8[:, 2 * kk : 2 * kk + 2, i * P : (i + 1) * P],
                            g8[kk][:, :, h * 512 : (h + 1) * 512],
                            start=(kk == 0),
                            stop=False,
                            perf_mode=DR,
                        )
                    if i % 2 == 1:
                        pair = (i - 1) // 2
                        if pair >= ulag:
                            u_and_stt(pair - ulag)
                for pair in range(RT // 2 - ulag, RT // 2):
                    u_and_stt(pair)

            rg = [list(range(CORES))]
            # initial norm pass with iter-0 transposes interleaved (PE is
            # otherwise idle while ACT/DVE normalize).
            for i in range(RT):
                norm_block(i, 0)
                if i >= TLAG:
                    transpose_block(i - TLAG)
            for b in range(RT - TLAG, RT):
                transpose_block(b)

            for it in range(DEPTH):
                s = it % 2
                arin_tr = dram.tile([512, 512], F8, name="arin_tr", tag="arin_tr")
                arout_tr = dram.tile([512, 512], F8, name="arout_tr", tag="arout_tr", addr_space="Shared")
                arin_br = dram.tile([512, 512], F8, name="arin_br", tag="arin_br")
                arout_br = dram.tile([512, 512], F8, name="arout_br", tag="arout_br", addr_space="Shared")
                arin_tl = dram.tile([512, 512], F8, name="arin_tl", tag="arin_tl")
                arout_tl = dram.tile([512, 512], F8, name="arout_tl", tag="arout_tl", addr_space="Shared")
                arin_p = dram.tile([OUT_DIM, DIM], F8, name="arin_p", tag="arin_p")
                arout_p = dram.tile([OUT_DIM, DIM], F8, name="arout_p", tag="arout_p", addr_space="Shared")

                drains = list("avavavav")
                phase_gram(range(DK // 2), 1, arin_tr, drains[:4], row0=0)
                nc.gpsimd.collective_compute(
                    "AllReduce", ALU.add,
                    ins=[arin_tr.opt()], outs=[arout_tr.opt()], replica_groups=rg,
                )
                phase_gram(range(DK // 2, DK), 1, arin_br, drains[4:], row0=DK // 2)
                nc.gpsimd.collective_compute(
                    "AllReduce", ALU.add,
                    ins=[arin_br.opt()], outs=[arout_br.opt()], replica_groups=rg,
                )
                # tl gram drains now; its AllReduce is emitted AFTER the PtX
                # one (uv h=1's U matmuls need p mid-loop; tl is only needed
                # at uv h=0). The transposes sit in the AllReduce latency
                # shadow (iter 0's ran interleaved with the initial norm).
                phase_gram(range(DK // 2), 0, arin_tl, list("avav"))
                if it > 0:
                    for b in range(RT):
                        transpose_block(b)
                phase_mlp(et8, b2e, AF.Exp)
                pts = phase_et_transpose(s)
                phase_p(pts, s)
                phase_ptx(arin_p)
                nc.gpsimd.collective_compute(
                    "AllReduce", ALU.add,
                    ins=[arin_p.opt()], outs=[arout_p.opt()], replica_groups=rg,
                )
                nc.gpsimd.collective_compute(
                    "AllReduce", ALU.add,
                    ins=[arin_tl.opt()], outs=[arout_tl.opt()], replica_groups=rg,
                )
                phase_er8t()
                # land AllReduce results
                for k in range(DK // 2):
                    nc.sync.dma_start(
                        g8[k // 2][:, k % 2, 512:1024],
                        arout_tr[k * P : (k + 1) * P, :],
                    )
                for k in range(DK // 2, DK):
                    nc.sync.dma_start(
                        g8[k // 2][:, k % 2, 512:1024],
                        arout_br[(k - DK // 2) * P : (k - DK // 2 + 1) * P, :],
                    )
                for k in range(DK // 2):
                    nc.sync.dma_start(
                        g8[k // 2][:, k % 2, 0:512],
                        arout_tl[k * P : (k + 1) * P, :],
                    )
                nc.sync.dma_start(ptx8[:OUT_DIM, :], arout_p[:, :])
                nc.sync.dma_start(ptx8[OUT_DIM : 2 * OUT_DIM, :], arout_p[:, :])
                phase_bl()
                phase_uv(1, s, ulag=3)
                so = (it + 1) % 2
                if it < DEPTH - 1:
                    def tail(i, it=it, s=s, so=so):
                        norm_stats(i, so)
                        if i >= 1:
                            quant_block(i - 1, so)
                    phase_uv(0, s, tail=tail)
                    quant_block(RT - 1, so)
                else:
                    # fb8 = fp8(SX * X_6) = fp8(S * SX*inv) into xb8 tiles
                    # (X_6 = S_6 * inv_5; xb8 is dead after this iter's
                    # gram/ptx so the tiles are recycled for the final MLP)
                    def fquant(i, s=s, so=so):
                        dst = xb8[i // 2][:, i % 2, :]
                        if i % 2 == 0:
                            nc.vector.tensor_scalar_mul(
                                dst, xr[i][:], sxv[so][:, i : i + 1]
                            )
                        else:
                            nc.scalar.activation(
                                dst, xr[i][:], AF.Copy,
                                scale=sxv[so][:, i : i + 1],
                            )
                    def tail(i, it=it, s=s, so=so):
                        nc.vector.tensor_scalar_mul(
                            sxv[so][:, i : i + 1], inv[s][:, i : i + 1], SX
                        )
                        if i >= 1:
                            fquant(i - 1)
                    phase_uv(0, s, tail=tail)
                    fquant(RT - 1)

            # final MLP in fp8 on the final state's transposes
            for b in range(RT):
                transpose_block(b)
            yt = xtp.tile([OUT_DIM, NS], F32, name="yt", tag="yt")
            phase_mlp(yt, b2, AF.Identity)
            # transpose Y.T -> rows and store
            yr = sqp.tile([P, RT, OUT_DIM], F32, name="yr", tag="sq")
            for j2 in range(2):
                pt = ps.tile([P, 512], F32, name="ps", tag="ps")
                for q in range(8):
                    i = 8 * j2 + q
                    nc.tensor.transpose(
                        pt[:, q * OUT_DIM : (q + 1) * OUT_DIM],
                        yt[:, i * P : (i + 1) * P].bitcast(F32),
                        identf[:OUT_DIM, :OUT_DIM],
                    )
                nc.vector.tensor_copy(yr[:, 8 * j2 : 8 * j2 + 8, :], pt[:])
            nc.sync.dma_start(
                y_ext.rearrange("(i p) o -> p i o", p=P), yr[:, :, :]
            )

    _split_waits(nc)
    return nc


_NC = None


def _get_nc():
    global _NC
    if _NC is None:
        _NC = _build()
    return _NC


def _q8(x):
    return np.clip(x, -240.0, 240.0).astype(ml_dtypes.float8_e4m3)


def _in_maps(X, W1, b1, W2, b2):
    X = np.asarray(X, dtype=np.float32)
    W1 = np.asarray(W1, dtype=np.float32)
    b1 = np.asarray(b1, dtype=np.float32).reshape(HIDDEN)
    W2 = np.asarray(W2, dtype=np.float32)
    b2c = np.asarray(b2, dtype=np.float32).reshape(OUT_DIM, 1)
    b2e = b2c + np.float32(math.log(SE))
    # w18: fp8(SW1*W1) packed [128, j, h] flat (non-interleaved; MLP1 runs
    # non-DR col-tiled)
    w18 = np.zeros((P, DK * HIDDEN), np.float32)
    for j in range(DK):
        w18[:, j * HIDDEN : (j + 1) * HIDDEN] = W1[j * P : (j + 1) * P, :] * SW1
    w18 = _q8(w18)
    # w2b4/b1s: replicated into partition groups 32q..32q+15 for the
    # col/row-tiled MLP
    w2b4 = np.zeros((P, OUT_DIM), np.float32)
    b1s = np.zeros((P, 1), np.float32)
    for q in range(4):
        w2b4[32 * q : 32 * q + HIDDEN, :] = W2
        b1s[32 * q : 32 * q + HIDDEN, 0] = b1
    w2b4 = w2b4.astype(ml_dtypes.bfloat16)
    Xb = X.astype(ml_dtypes.bfloat16)
    return [
        {
            "x": np.ascontiguousarray(Xb[c * NS : (c + 1) * NS]),
            "w18": w18,
            "w2b4": w2b4,
            "b1s": b1s,
            "b2": b2c,
            "b2e": b2e,
        }
        for c in range(CORES)
    ]


def run(X, W1, b1, W2, b2, **kwargs):
    nc = _get_nc()
    res = run_bass_kernel_spmd(nc, _in_maps(X, W1, b1, W2, b2), list(range(CORES)), **kwargs)
    out = np.concatenate([res.results[c]["y"] for c in range(CORES)], axis=0)
    return out, res


def kernel(X, W1, b1, W2, b2):
    out, _ = run(X, W1, b1, W2, b2)
    return out


# revision 32
# speedup vs baseline: 1.0986x; 1.0145x over previous
"""CKAFormer Trainium2 kernel, fp8 edition.

6 iterations of
    Xn = X / ||X||_row;  P = softmax(relu(Xn@W1+b1)@W2+b2)
    X  = Xn + g*P@(P.T@Xn) - g*Xn@(Xn.T@Xn)
then a final MLP. Row-sharded over 8 NeuronCores.

Speed scheme vs the bf16 baseline:
- State is kept per-row-SCALED (S = nrm*X): the row normalization of the
  leading term cancels. True X is recovered by one in-place scale pass in
  the last iteration only.
- U and V updates accumulate into ONE PSUM bank: with g8 = -8*G,
  er8 = 16*P, ptx8 = 8*PtX and xt8 = 16*Xn^T, both terms come out as
  128*(P@PtX - Xn@G), so a single scalar_tensor_tensor with per-row
  scalar sd*GAMMA/128 applies the whole update.
- Big in-loop matmuls (Gram, V, PtX) run fp8e4m3 DoubleRow (2 k-blocks
  per pass). The U matmuls (K=64) run pairwise-concurrent via
  tile_position row tiling (er8t/ptx8 mirrored to partitions 64-127).
- MLP1 (M=16) runs as 4 concurrent col-tiled chains (col groups 0..3,
  non-DR fp8, issued j-major); MLP2 (K=16) as 4 concurrent row-tiled
  matmuls. w2b/b1 are host-replicated across the 4 partition groups.
- Xn^T transposes run as a dense phase in the AllReduce latency shadow
  (right after the gram drains): 8 stride-2 transposes per block packed
  into one PSUM bank, one multi-dim drain copy, ACT/DVE alternating.
- The uv(h=0) tail is norm-only: sq+accum and sqrt on ACT, reciprocal/
  sxv/svc on DVE, and the fp8 re-quantize lagged one block (QLAG) so the
  strict-FIFO engine queues never head-of-line block on the stats chain.
- State X is held in bf16 (the update enters through GAMMA=1e-4; bf16
  rounding is ~8x below the fp8 operand noise already present).
- G is AllReduced in fp8 as top-right + bottom-right + top-left [512,512]
  chunks; the bottom-left quadrant is reconstructed locally as
  transpose(top-right). PtX is a fourth fp8 AllReduce. Update error
  enters X only through GAMMA=1e-4.
- Element-wise work is split across DVE and ACT only (GpSimd ucode
  tensor ops are ~10x slower and cannot touch PSUM).
The final MLP also runs fp8, reusing w18 and the tail transposes of the
last iteration's output.
"""

import sys

sys.path.insert(0, "/opt/trn_rl_repo")

import math

import ml_dtypes
import numpy as np

import concourse.bass as bass
import concourse.mybir as mybir
import concourse.tile as tile
from concourse.bass_utils import run_bass_kernel_spmd
from concourse.masks import make_identity
from concourse.vector_clock import ScopedClock

DEPTH = 6
GAMMA = 1e-4
DIM = 1024
HIDDEN = 16
OUT_DIM = 64
N = 16384
CORES = 8

NS = N // CORES        # rows per core = 2048
RT = NS // 128         # row tiles = 16
DK = DIM // 128        # dim k-tiles = 8
P = 128

F32 = mybir.dt.float32
F32R = mybir.dt.float32r
BF = mybir.dt.bfloat16
F8 = mybir.dt.float8e4
AF = mybir.ActivationFunctionType
ALU = mybir.AluOpType
DR = mybir.MatmulPerfMode.DoubleRow

SX = 16.0    # xb8/xt8 = SX * Xn
SW1 = 32.0   # w18 = SW1 * W1
SP = 16.0    # er8/er8t = SP * P
SE = 4.0     # et8 = SE * E
SG = 8.0     # g8 wire = -SG * G ;  ptx8 wire = SG * PtX

GRAM_DRAIN = -SG / (SX * SX)           # psum(SX^2 G) -> -8*G
PTX_DRAIN = SG / (SP * SX)             # psum(SP*SX*PtX) -> 8*PtX
MLP1_SCALE = 1.0 / (SX * SW1)          # psum -> Xn@W1
CUV = GAMMA / (SX * SG)                # fused STT: svc = sd*GAMMA/128

TLAG = 6  # tail transposes run this many blocks behind the uv update

# this container's walrus only accepts one sync-wait slot per engine
# instruction; hoist excess waits onto preceding EventSemaphore carriers.
_MAX_WAITS = 1


class _TC(tile.TileContext):
    def _drain_and_barrier(self, tick_clock, wait_clock):
        drain_inst = self.nc.sync.drain()
        wait_clock.add_sem_waits(
            drain_inst.ins, ScopedClock({None: tick_clock.global_clock})
        )
        si = drain_inst.ins.sync_info
        w = list(si.on_wait) if si and si.on_wait else []
        if len(w) > 1:
            si.on_wait = w[:1]
            for i in range(1, len(w)):
                c = self.nc.sync.drain()
                c.ins.sync_info = mybir.SyncInfo(on_wait=[w[i]], on_update=[])
        self.nc.all_engine_barrier()
        assert self.sems is not None
        popped = self.nc._tile_sem_poison_stack.pop()
        assert popped is self._sem_poison
        self.nc.clear_and_free_semaphores(list(self.sems.allocated().values()))
        self.nc.all_engine_barrier()


def _split_waits(nc, limit=_MAX_WAITS):
    """Hoist excess sem waits onto EventSemaphore carriers inserted just
    before the over-limit instruction (per-engine program order preserves the
    gating; waits are a conjunction so splitting is sound)."""
    nid = 0
    for bb in nc.main_func.blocks:
        out = []
        changed = False
        for ins in bb.instructions:
            si = ins.sync_info
            w = list(si.on_wait) if si and si.on_wait else []
            if len(w) > limit:
                extra, keep = w[:-limit], w[-limit:]
                for i in range(0, len(extra), limit):
                    ev = mybir.InstEventSemaphore(name=f"wsplit_{nid}", ins=[], outs=[])
                    nid += 1
                    ev.engine = ins.engine
                    ev.sync_info = mybir.SyncInfo(
                        on_wait=extra[i : i + limit], on_update=[]
                    )
                    out.append(ev)
                si.on_wait = keep
                changed = True
            out.append(ins)
        if changed:
            bb.instructions = out


def _build():
    nc = bass.Bass()
    x_ext = nc.declare_dram_parameter("x", [NS, DIM], BF, isOutput=False)
    w18_ext = nc.declare_dram_parameter("w18", [P, DK * HIDDEN], F8, isOutput=False)
    w2b4_ext = nc.declare_dram_parameter("w2b4", [P, OUT_DIM], BF, isOutput=False)
    b1s_ext = nc.declare_dram_parameter("b1s", [P, 1], F32, isOutput=False)
    b2_ext = nc.declare_dram_parameter("b2", [OUT_DIM, 1], F32, isOutput=False)
    b2e_ext = nc.declare_dram_parameter("b2e", [OUT_DIM, 1], F32, isOutput=False)
    y_ext = nc.declare_dram_parameter("y", [NS, OUT_DIM], F32, isOutput=True)

    with _TC(nc) as tc:
        with (
            tc.tile_pool(name="state", bufs=1) as st,
            tc.tile_pool(name="sq", bufs=1) as sqp,
            tc.tile_pool(name="stg", bufs=6) as stg,
            tc.tile_pool(name="xtmp", bufs=1) as xtp,
            tc.tile_pool(name="ps", bufs=8, space="PSUM") as ps,
            tc.tile_pool(name="dram", bufs=2, space="DRAM") as dram,
        ):
            # persistent state
            xr = [st.tile([P, DIM], BF, name=f"xr{i}", tag=f"xr{i}") for i in range(RT)]
            xb8 = [st.tile([P, 2, DIM], F8, name=f"xb8{j}", tag=f"xb8{j}") for j in range(DK)]
            xt8 = st.tile([P, DK, NS], F8, name="xt8", tag="xt8")
            g8 = [st.tile([P, 2, DIM], F8, name=f"g8{k}", tag=f"g8{k}") for k in range(DK // 2)]
            et8 = st.tile([OUT_DIM, NS], F8, name="et8", tag="et8")
            er8 = st.tile([P, RT, OUT_DIM], F8, name="er8", tag="er8")
            er8t = st.tile([P, NS], F8, name="er8t", tag="er8t")
            ptx8 = st.tile([P, DIM], F8, name="ptx8", tag="ptx8")
            a1 = st.tile([P, 512], BF, name="a1", tag="a1")
            w18 = st.tile([P, DK * HIDDEN], F8, name="w18", tag="w18")
            w2b4 = st.tile([P, OUT_DIM], BF, name="w2b4", tag="w2b4")
            b1s = st.tile([P, 1], F32, name="b1s", tag="b1s")
            b2 = st.tile([OUT_DIM, 1], F32, name="b2", tag="b2")
            b2e = st.tile([OUT_DIM, 1], F32, name="b2e", tag="b2e")
            ident8 = st.tile([P, P], F8, name="ident8", tag="ident8")
            identf = st.tile([P, P], F32, name="identf", tag="identf")
            # per-iteration stats, double-buffered across iterations
            n2 = [st.tile([P, RT], F32, name=f"n2{s}", tag=f"n2{s}") for s in range(2)]
            sd = [st.tile([P, RT], F32, name=f"sd{s}", tag=f"sd{s}") for s in range(2)]
            inv = [st.tile([P, RT], F32, name=f"inv{s}", tag=f"inv{s}") for s in range(2)]
            sxv = [st.tile([P, RT], F32, name=f"sxv{s}", tag=f"sxv{s}") for s in range(2)]
            srow = [st.tile([P, RT], F32, name=f"srow{s}", tag=f"srow{s}") for s in range(2)]
            s16 = [st.tile([P, RT], F32, name=f"s16{s}", tag=f"s16{s}") for s in range(2)]
            svc = [st.tile([P, RT], F32, name=f"svc{s}", tag=f"svc{s}") for s in range(2)]

            # loads
            for i in range(RT):
                nc.sync.dma_start(xr[i][:], x_ext[i * P : (i + 1) * P, :])
            nc.sync.dma_start(w18[:], w18_ext[:, :])
            nc.sync.dma_start(w2b4[:], w2b4_ext[:, :])
            nc.sync.dma_start(b1s[:], b1s_ext[:, :])
            nc.sync.dma_start(b2[:], b2_ext[:, :])
            nc.sync.dma_start(b2e[:], b2e_ext[:, :])
            make_identity(nc, identf[:])
            nc.vector.tensor_copy(ident8[:], identf[:])

            def norm_stats(i, s):
                # row norm stats of (raw) block i into stats set s:
                # sq+accum, sqrt on ACT; reciprocal, sxv, fused svc on DVE.
                sq = sqp.tile([P, DIM], BF, name="sq", tag="sq")
                nc.scalar.activation(
                    sq[:], xr[i][:], AF.Square, accum_out=n2[s][:, i : i + 1]
                )
                nc.scalar.sqrt(sd[s][:, i : i + 1], n2[s][:, i : i + 1])
                nc.vector.reciprocal(inv[s][:, i : i + 1], sd[s][:, i : i + 1])
                nc.vector.tensor_scalar_mul(
                    sxv[s][:, i : i + 1], inv[s][:, i : i + 1], SX
                )
                nc.vector.tensor_scalar_mul(
                    svc[s][:, i : i + 1], sd[s][:, i : i + 1], CUV
                )

            def quant_block(i, s):
                # xb8 <- fp8(SX * Xn); issued one block late (QLAG) so the
                # engine queues never head-of-line block on the stats chain.
                dst = xb8[i // 2][:, i % 2, :]
                if i % 2 == 0:
                    nc.vector.tensor_scalar_mul(dst, xr[i][:], sxv[s][:, i : i + 1])
                else:
                    nc.scalar.activation(
                        dst, xr[i][:], AF.Copy, scale=sxv[s][:, i : i + 1]
                    )

            def norm_block(i, s):
                norm_stats(i, s)
                quant_block(i, s)

            def transpose_block(b):
                # xt8[:, k, b*128:(b+1)*128] = fp8(SX*Xn[b-block, :].T)
                # 8 stride-2 transposes packed into one PSUM bank, one
                # multi-dim copy out (engine alternates by block parity).
                pt = ps.tile([P, 2048], F8, name="pstb", tag="ps")
                for k in range(DK):
                    nc.tensor.transpose(
                        pt[:, k * 2 * P : (k + 1) * 2 * P : 2],
                        xb8[b // 2][:, b % 2, k * P : (k + 1) * P],
                        ident8[:],
                    )
                dst = xt8[:, :, b * P : (b + 1) * P]
                if b % 2 == 0:
                    nc.scalar.copy(dst, pt[:, 0:2048:2])
                else:
                    nc.vector.tensor_copy(dst, pt[:, 0:2048:2])

            def phase_gram(ms, h, arin, drain_rr, row0=0):
                # partial (SX Xn).T @ (SX Xn) over row tiles for m-blocks `ms`,
                # column half h; drain scaled to -8*G fp8 into arin rows
                # (m-row0)*128.
                for m in ms:
                    pg = ps.tile([P, 512], F32, name="ps", tag="ps")
                    for j in range(DK):
                        nc.tensor.matmul(
                            pg[:],
                            xb8[j][:, :, m * P : (m + 1) * P],
                            xb8[j][:, :, h * 512 : (h + 1) * 512],
                            start=(j == 0),
                            stop=(j == DK - 1),
                            perf_mode=DR,
                        )
                    gs = stg.tile([P, 512], F8, name="gs", tag="gs")
                    if drain_rr.pop(0) == "a":
                        nc.scalar.mul(gs[:], pg[:], GRAM_DRAIN)
                    else:
                        nc.vector.tensor_scalar_mul(gs[:], pg[:], GRAM_DRAIN)
                    nc.sync.dma_start(arin[(m - row0) * P : (m - row0 + 1) * P, :], gs[:])

            def phase_mlp(et_dst, bias, act_fn):
                # 4 col-tiled MLP1 chains (M=16 in col groups 0..3, non-DR
                # fp8) issued j-major so the four chains run concurrently in
                # the array, + 4 row-tiled MLP2 matmuls (K=16 in row groups).
                pa = ps.tile([P, 512], F32, name="ps", tag="ps")
                for j in range(DK):
                    for q in range(4):
                        nc.tensor.matmul(
                            pa[32 * q : 32 * q + HIDDEN, :],
                            w18[:, j * HIDDEN : (j + 1) * HIDDEN],
                            xt8[:, j, q * 512 : (q + 1) * 512],
                            start=(j == 0),
                            stop=(j == DK - 1),
                            tile_position=(0, 32 * q),
                        )
                for q in range(4):
                    nc.scalar.activation(
                        a1[32 * q : 32 * q + HIDDEN, :],
                        pa[32 * q : 32 * q + HIDDEN, :],
                        AF.Relu,
                        bias=b1s[32 * q : 32 * q + HIDDEN, :],
                        scale=MLP1_SCALE,
                    )
                pbs = []
                for q in range(4):
                    pb = ps.tile([OUT_DIM, 512], F32, name="ps2", tag="ps")
                    nc.tensor.matmul(
                        pb[:],
                        w2b4[32 * q : 32 * q + HIDDEN, :],
                        a1[32 * q : 32 * q + HIDDEN, :],
                        tile_position=(32 * q, 0),
                    )
                    pbs.append(pb)
                for q in range(4):
                    sl = slice(q * 512, (q + 1) * 512)
                    nc.scalar.activation(
                        et_dst[:, sl], pbs[q][:], act_fn, bias=bias[:]
                    )

            def phase_et_transpose(s):
                # transpose et8 (4E) to rows (stride-2 fp8); srow = sum(4E)
                pts = []
                for j2 in range(2):
                    pt = ps.tile([P, 1024], F8, name="ps8", tag="ps")
                    for q in range(8):
                        i = 8 * j2 + q
                        nc.tensor.transpose(
                            pt[:, q * 2 * OUT_DIM : (q + 1) * 2 * OUT_DIM : 2],
                            et8[:, i * P : (i + 1) * P],
                            ident8[:OUT_DIM, :OUT_DIM],
                        )
                    nc.vector.tensor_reduce(
                        srow[s][:, 8 * j2 : 8 * j2 + 8],
                        pt[:].rearrange("p (i o t) -> p i o t", o=OUT_DIM, t=2)[:, :, :, 0],
                        mybir.AxisListType.X,
                        ALU.add,
                    )
                    pts.append(pt)
                return pts

            def phase_p(pts, s):
                # er8[:, i, :] = fp8(SP * P-rows) = pt * s16 (split ACT/DVE)
                nc.vector.reciprocal(s16[s][:], srow[s][:])
                nc.vector.tensor_scalar_mul(s16[s][:], s16[s][:], SP)
                for j2 in range(2):
                    for q in range(8):
                        i = 8 * j2 + q
                        src = pts[j2][:, q * 2 * OUT_DIM : (q + 1) * 2 * OUT_DIM : 2]
                        if i % 2 == 0:
                            nc.vector.tensor_scalar_mul(
                                er8[:, i, :], src, s16[s][:, i : i + 1]
                            )
                        else:
                            nc.scalar.mul(
                                er8[:, i, :], src, s16[s][:, i : i + 1]
                            )

            def phase_er8t():
                # er8t = (16P).T via PE transposes of er8 rows; mirrored to
                # partitions 64-127 with one SBUF->SBUF DMA for U row-tiling.
                for j2 in range(2):
                    pt = ps.tile([OUT_DIM, 2048], F8, name="pse", tag="ps")
                    for q in range(8):
                        i = 8 * j2 + q
                        nc.tensor.transpose(
                            pt[:, q * 2 * P : (q + 1) * 2 * P : 2],
                            er8[:, i, :],
                            ident8[:],
                        )
                    if j2 == 0:
                        nc.scalar.copy(
                            er8t[:OUT_DIM, j2 * 1024 : (j2 + 1) * 1024], pt[:, 0:2048:2]
                        )
                    else:
                        nc.vector.tensor_copy(
                            er8t[:OUT_DIM, j2 * 1024 : (j2 + 1) * 1024], pt[:, 0:2048:2]
                        )
                nc.sync.dma_start(er8t[OUT_DIM : 2 * OUT_DIM, :], er8t[:OUT_DIM, :])

            def phase_ptx(arin):
                # partial (SP*P).T @ (SX*Xn) -> fp8(SG*PtX) wire
                for h in range(2):
                    pp = ps.tile([OUT_DIM, 512], F32, name="ps", tag="ps")
                    for j in range(DK):
                        nc.tensor.matmul(
                            pp[:],
                            er8[:, 2 * j : 2 * j + 2, :],
                            xb8[j][:, :, h * 512 : (h + 1) * 512],
                            start=(j == 0),
                            stop=(j == DK - 1),
                            perf_mode=DR,
                        )
                    pps = stg.tile([OUT_DIM, 512], F8, name="pps", tag="gs")
                    nc.scalar.mul(pps[:], pp[:], PTX_DRAIN)
                    nc.sync.dma_start(arin[:, h * 512 : (h + 1) * 512], pps[:])

            def phase_bl():
                # bottom-left of g8 = transpose(top-right): g8 cols 0:512 for
                # k-blocks 4..7 from g8 cols 512:1024 of k-blocks 0..3.
                for b in range(4):
                    pt = ps.tile([P, 1024], F8, name="ps8", tag="ps")
                    for a in range(4):
                        nc.tensor.transpose(
                            pt[:, a * 2 * P : (a + 1) * 2 * P : 2],
                            g8[a // 2][:, a % 2, 512 + b * P : 512 + (b + 1) * P],
                            ident8[:],
                        )
                    if b % 2 == 0:
                        nc.scalar.copy(g8[2 + b // 2][:, b % 2, 0:512], pt[:, 0:1024:2])
                    else:
                        nc.vector.tensor_copy(
                            g8[2 + b // 2][:, b % 2, 0:512], pt[:, 0:1024:2]
                        )

            def phase_uv(h, s, tail=None, ulag=0):
                # per block: fused PSUM chain 128*(P@PtX - Xn@G) cols h via
                # 4 DR matmuls + a U matmul; U matmuls run pairwise
                # concurrent via row tiles (0,0)/(64,0) using the er8t/ptx8
                # partition mirrors. Then one STT: xsl += svc * psum.
                # ulag defers each pair's U+STT by that many pairs so the
                # U matmuls never wait on the PtX AllReduce landing.
                pus = {}
                def u_and_stt(pair):
                    for ii, tp in ((2 * pair, 0), (2 * pair + 1, OUT_DIM)):
                        nc.tensor.matmul(
                            pus[ii][:],
                            er8t[tp : tp + OUT_DIM, ii * P : (ii + 1) * P],
                            ptx8[tp : tp + OUT_DIM, h * 512 : (h + 1) * 512],
                            start=False,
                            stop=True,
                            tile_position=(tp, 0),
                        )
                    for ii in (2 * pair, 2 * pair + 1):
                        xsl = xr[ii][:, h * 512 : (h + 1) * 512]
                        nc.vector.scalar_tensor_tensor(
                            xsl, pus.pop(ii)[:], svc[s][:, ii : ii + 1], xsl,
                            ALU.mult, ALU.add,
                        )
                        if tail is not None:
                            tail(ii)
                for i in range(RT):
                    pu = ps.tile([P, 512], F32, name="ps", tag="ps")
                    pus[i] = pu
                    for kk in range(DK // 2):
                        nc.tensor.matmul(
                            pu[:],
                            xt8[:, 2 * kk : 2 * kk + 2, i * P : (i + 1) * P],
                            g8[kk][:, :, h * 512 : (h + 1) * 512],
                            start=(kk == 0),
                            stop=False,
                            perf_mode=DR,
                        )
                    if i % 2 == 1:
                        pair = (i - 1) // 2
                        if pair >= ulag:
                            u_and_stt(pair - ulag)
                for pair in range(RT // 2 - ulag, RT // 2):
                    u_and_stt(pair)

            rg = [list(range(CORES))]
            for i in range(RT):
                norm_block(i, 0)

            for it in range(DEPTH):
                s = it % 2
                arin_tr = dram.tile([512, 512], F8, name="arin_tr", tag="arin_tr")
                arout_tr = dram.tile([512, 512], F8, name="arout_tr", tag="arout_tr", addr_space="Shared")
                arin_br = dram.tile([512, 512], F8, name="arin_br", tag="arin_br")
                arout_br = dram.tile([512, 512], F8, name="arout_br", tag="arout_br", addr_space="Shared")
                arin_tl = dram.tile([512, 512], F8, name="arin_tl", tag="arin_tl")
                arout_tl = dram.tile([512, 512], F8, name="arout_tl", tag="arout_tl", addr_space="Shared")
                arin_p = dram.tile([OUT_DIM, DIM], F8, name="arin_p", tag="arin_p")
                arout_p = dram.tile([OUT_DIM, DIM], F8, name="arout_p", tag="arout_p", addr_space="Shared")

                drains = list("avavavav")
                phase_gram(range(DK // 2), 1, arin_tr, drains[:4], row0=0)
                nc.gpsimd.collective_compute(
                    "AllReduce", ALU.add,
                    ins=[arin_tr.opt()], outs=[arout_tr.opt()], replica_groups=rg,
                )
                phase_gram(range(DK // 2, DK), 1, arin_br, drains[4:], row0=DK // 2)
                nc.gpsimd.collective_compute(
                    "AllReduce", ALU.add,
                    ins=[arin_br.opt()], outs=[arout_br.opt()], replica_groups=rg,
                )
                # tl gram drains now; its AllReduce is emitted AFTER the PtX
                # one (uv h=1's U matmuls need p mid-loop; tl is only needed
                # at uv h=0). The transposes sit in the AllReduce latency
                # shadow (iter 0's ran interleaved with the initial norm).
                phase_gram(range(DK // 2), 0, arin_tl, list("avav"))
                for b in range(RT):
                    transpose_block(b)
                phase_mlp(et8, b2e, AF.Exp)
                pts = phase_et_transpose(s)
                phase_p(pts, s)
                phase_ptx(arin_p)
                nc.gpsimd.collective_compute(
                    "AllReduce", ALU.add,
                    ins=[arin_p.opt()], outs=[arout_p.opt()], replica_groups=rg,
                )
                nc.gpsimd.collective_compute(
                    "AllReduce", ALU.add,
                    ins=[arin_tl.opt()], outs=[arout_tl.opt()], replica_groups=rg,
                )
                phase_er8t()
                # land AllReduce results
                for k in range(DK // 2):
                    nc.sync.dma_start(
                        g8[k // 2][:, k % 2, 512:1024],
                        arout_tr[k * P : (k + 1) * P, :],
                    )
                for k in range(DK // 2, DK):
                    nc.sync.dma_start(
                        g8[k // 2][:, k % 2, 512:1024],
                        arout_br[(k - DK // 2) * P : (k - DK // 2 + 1) * P, :],
                    )
                for k in range(DK // 2):
                    nc.sync.dma_start(
                        g8[k // 2][:, k % 2, 0:512],
                        arout_tl[k * P : (k + 1) * P, :],
                    )
                nc.sync.dma_start(ptx8[:OUT_DIM, :], arout_p[:, :])
                nc.sync.dma_start(ptx8[OUT_DIM : 2 * OUT_DIM, :], arout_p[:, :])
                phase_bl()
                phase_uv(1, s)
                so = (it + 1) % 2
                if it < DEPTH - 1:
                    def tail(i, it=it, s=s, so=so):
                        norm_stats(i, so)
                        if i >= 1:
                            quant_block(i - 1, so)
                    phase_uv(0, s, tail=tail)
                    quant_block(RT - 1, so)
                else:
                    # fb8 = fp8(SX * X_6) = fp8(S * SX*inv) into xb8 tiles
                    # (X_6 = S_6 * inv_5; xb8 is dead after this iter's
                    # gram/ptx so the tiles are recycled for the final MLP)
                    def fquant(i, s=s, so=so):
                        dst = xb8[i // 2][:, i % 2, :]
                        if i % 2 == 0:
                            nc.vector.tensor_scalar_mul(
                                dst, xr[i][:], sxv[so][:, i : i + 1]
                            )
                        else:
                            nc.scalar.activation(
                                dst, xr[i][:], AF.Copy,
                                scale=sxv[so][:, i : i + 1],
                            )
                    def tail(i, it=it, s=s, so=so):
                        nc.vector.tensor_scalar_mul(
                            sxv[so][:, i : i + 1], inv[s][:, i : i + 1], SX
                        )
                        if i >= 1:
                            fquant(i - 1)
                    phase_uv(0, s, tail=tail)
                    fquant(RT - 1)

            # final MLP in fp8 on the final state's transposes
            for b in range(RT):
                transpose_block(b)
            yt = xtp.tile([OUT_DIM, NS], F32, name="yt", tag="yt")
            phase_mlp(yt, b2, AF.Identity)
            # transpose Y.T -> rows and store
            yr = sqp.tile([P, RT, OUT_DIM], F32, name="yr", tag="sq")
            for j2 in range(2):
                pt = ps.tile([P, 512], F32, name="ps", tag="ps")
                for q in range(8):
                    i = 8 * j2 + q
                    nc.tensor.transpose(
                        pt[:, q * OUT_DIM : (q + 1) * OUT_DIM],
                        yt[:, i * P : (i + 1) * P].bitcast(F32),
                        identf[:OUT_DIM, :OUT_DIM],
                    )
                nc.vector.tensor_copy(yr[:, 8 * j2 : 8 * j2 + 8, :], pt[:])
            nc.sync.dma_start(
                y_ext.rearrange("(i p) o -> p i o", p=P), yr[:, :, :]
            )

    _split_waits(nc)
    return nc


_NC = None


def _get_nc():
    global _NC
    if _NC is None:
        _NC = _build()
    return _NC


def _q8(x):
    return np.clip(x, -240.0, 240.0).astype(ml_dtypes.float8_e4m3)


def _in_maps(X, W1, b1, W2, b2):
    X = np.asarray(X, dtype=np.float32)
    W1 = np.asarray(W1, dtype=np.float32)
    b1 = np.asarray(b1, dtype=np.float32).reshape(HIDDEN)
    W2 = np.asarray(W2, dtype=np.float32)
    b2c = np.asarray(b2, dtype=np.float32).reshape(OUT_DIM, 1)
    b2e = b2c + np.float32(math.log(SE))
    # w18: fp8(SW1*W1) packed [128, j, h] flat (non-interleaved; MLP1 runs
    # non-DR col-tiled)
    w18 = np.zeros((P, DK * HIDDEN), np.float32)
    for j in range(DK):
        w18[:, j * HIDDEN : (j + 1) * HIDDEN] = W1[j * P : (j + 1) * P, :] * SW1
    w18 = _q8(w18)
    # w2b4/b1s: replicated into partition groups 32q..32q+15 for the
    # col/row-tiled MLP
    w2b4 = np.zeros((P, OUT_DIM), np.float32)
    b1s = np.zeros((P, 1), np.float32)
    for q in range(4):
        w2b4[32 * q : 32 * q + HIDDEN, :] = W2
        b1s[32 * q : 32 * q + HIDDEN, 0] = b1
    w2b4 = w2b4.astype(ml_dtypes.bfloat16)
    Xb = X.astype(ml_dtypes.bfloat16)
    return [
        {
            "x": np.ascontiguousarray(Xb[c * NS : (c + 1) * NS]),
            "w18": w18,
            "w2b4": w2b4,
            "b1s": b1s,
            "b2": b2c,
            "b2e": b2e,
        }
        for c in range(CORES)
    ]


def run(X, W1, b1, W2, b2, **kwargs):
    nc = _get_nc()
    res = run_bass_kernel_spmd(nc, _in_maps(X, W1, b1, W2, b2), list(range(CORES)), **kwargs)
    out = np.concatenate([res.results[c]["y"] for c in range(CORES)], axis=0)
    return out, res


def kernel(X, W1, b1, W2, b2):
    out, _ = run(X, W1, b1, W2, b2)
    return out
